# revision 48
# baseline (speedup 1.0000x reference)
"""Trainium2 Bass kernel for nn_Encoder_Block (B=2,S=2048,D=1024,H=16,FF=4096).

Sharding: 8 cores, core c -> (batch b=c//4, query block q=c%4 of 512 tokens).
Each core recomputes K/V for its whole batch (no cross-core collectives),
everything else is perfectly sharded. Host does transposes and gather.

Device layout: activations kept transposed [feature, token] throughout, so
every matmul in the chain is a natural lhsT/rhs pair with K=128 contraction
chunks and N=512 moving dim. Attention computes transposed scores [t, sq];
softmax normalizer rides along the PV matmul as a ones-column in V (M=65).
Masking + 1/sqrt(dh) scaling are folded into the Exp activation (bias/scale).
No max-subtraction: scores are O(1) by construction, exp is safe in fp32.

v2: bf16 matmul path, fully-masked key chunks skipped (program specialized
on ceil(max(len)/128) at runtime), dedicated early weight prefetch, fast
approx reciprocal, trivial-LN fast path (g==1, b==0 checked host-side),
broadcasts done as K=1 matmuls on the (otherwise idle) PE.
"""
import sys, types, os
sys.path.insert(0, "/opt/trn_rl_repo")
import numpy as np
from contextlib import ExitStack

import concourse.bass as bass
import concourse.tile as tile
from concourse import bacc, mybir
from concourse.bass_utils import run_bass_kernel_spmd

B, S, D, H, FF = 2, 2048, 1024, 16, 4096
DH = D // H            # 64
SQ = 512               # query tokens per core
NCORES = 8
EPS = 1e-5
MASK_NEG = -60.0       # exp(-60) ~ 8.8e-27 => masked keys contribute ~0

F32 = mybir.dt.float32
F32R = mybir.dt.float32r
# PE compute dtype for matmul-fed tensors:
#   bf16 : 1 cyc/row, half DMA/LDWEIGHTS cost, ~5e-3 output error
#   f32r : 1 cyc/row TF32-like, ~3.5e-4 output error
_MODE = os.environ.get("KERNEL_MM_DT", "bf16")
DT = {"f32r": mybir.dt.float32r, "f32": mybir.dt.float32,
      "bf16": mybir.dt.bfloat16}[_MODE]


def _f(ap):
    # f32r tiles must be bitcast to f32 for DVE/ACT reads; bf16 is native
    if DT == mybir.dt.float32r:
        return ap.bitcast(F32)
    return ap


def _install_ntff_hook():
    """The image's antenv lacks axon_hooks; shim it so trace=True works."""
    try:
        import antenv.axon_hooks  # noqa
        return
    except ImportError:
        pass
    try:
        from trn_agent_boot.trn_boot import _ntff_profile_via_ctypes
        import antenv
        mod = types.ModuleType("antenv.axon_hooks")
        hook = _ntff_profile_via_ctypes("/opt/axon/libaxon_pjrt.so")
        mod.get_axon_ntff_profile_hook = lambda: hook
        mod.set_axon_ntff_profile_hook = lambda h: None
        sys.modules["antenv.axon_hooks"] = mod
        antenv.axon_hooks = mod
    except Exception:
        pass


def _mm(nc, out, lhsT, rhs, start, stop, tile_position=None):
    nc.tensor.matmul(out, lhsT, rhs,
                     start=start, stop=stop, tile_position=tile_position)


def build_nc(nact=16, triv=True):
    """nact: number of active 128-key chunks (fully-masked tail skipped).
    triv: all LN gammas are 1, all betas/biases 0 (checked host-side)."""
    nc = bacc.Bacc(trn_type="TRN2", target_bir_lowering=False, debug=False,
                   num_devices=NCORES, dynamic_dma_scratch_size=512)
    AF = mybir.ActivationFunctionType
    OP = mybir.AluOpType

    # sc groups of up to 4 active 128-token chunks each
    SCG = []
    rem = nact
    while rem > 0:
        SCG.append(min(4, rem))
        rem -= 4

    # ---- DRAM I/O (per-core; program identical across cores) ----
    d_xT = nc.dram_tensor("xT", [D, S], DT, kind="ExternalInput")
    d_xq = nc.dram_tensor("xq", [D, SQ], DT, kind="ExternalInput")
    d_mask = nc.dram_tensor("maskb", [128, S // 128], F32, kind="ExternalInput")
    d_wq = nc.dram_tensor("wq", [D, D], DT, kind="ExternalInput")
    d_wk = nc.dram_tensor("wk", [D, D], DT, kind="ExternalInput")
    d_wv = nc.dram_tensor("wv", [D, D], DT, kind="ExternalInput")
    d_wo = nc.dram_tensor("wo", [D, D], DT, kind="ExternalInput")
    d_aw1 = nc.dram_tensor("aw1", [D, D], DT, kind="ExternalInput")
    d_aw2 = nc.dram_tensor("aw2", [D, D], DT, kind="ExternalInput")
    d_fw1 = nc.dram_tensor("fw1", [D, FF], DT, kind="ExternalInput")
    d_fw2 = nc.dram_tensor("fw2", [FF, D], DT, kind="ExternalInput")
    d_b1 = nc.dram_tensor("b1c", [128, 8], F32, kind="ExternalInput")
    d_g1 = nc.dram_tensor("g1c", [128, 8], F32, kind="ExternalInput")
    d_bb1 = nc.dram_tensor("bb1c", [128, 8], F32, kind="ExternalInput")
    d_fb1 = nc.dram_tensor("fb1c", [128, 32], F32, kind="ExternalInput")
    d_fb2 = nc.dram_tensor("fb2c", [128, 8], F32, kind="ExternalInput")
    d_b2 = nc.dram_tensor("b2c", [128, 8], F32, kind="ExternalInput")
    d_g2 = nc.dram_tensor("g2c", [128, 8], F32, kind="ExternalInput")
    d_bb2 = nc.dram_tensor("bb2c", [128, 8], F32, kind="ExternalInput")
    d_out = nc.dram_tensor("out", [D, SQ], DT, kind="ExternalOutput")

    r_xT = d_xT.ap().rearrange("(c p) s -> p c s", p=128)     # [128, 8, S]
    r_xq = d_xq.ap().rearrange("(c p) s -> p c s", p=128)     # [128, 8, SQ]
    r_wq = d_wq.ap().rearrange("(c p) n -> p c n", p=128)
    r_wk = d_wk.ap().rearrange("(c p) n -> p c n", p=128)
    r_wv = d_wv.ap().rearrange("(c p) n -> p c n", p=128)
    r_wo = d_wo.ap().rearrange("(c p) n -> p c n", p=128)
    r_aw1 = d_aw1.ap().rearrange("(c p) n -> p c n", p=128)
    r_aw2 = d_aw2.ap().rearrange("(c p) n -> p c n", p=128)
    r_fw1 = d_fw1.ap().rearrange("(c p) n -> p c n", p=128)   # [128, 8, FF]
    r_fw2 = d_fw2.ap().rearrange("(c p) n -> p c n", p=128)   # [128, 32, D]
    r_out = d_out.ap().rearrange("(c p) s -> p c s", p=128)

    with tile.TileContext(nc) as tc:
      with ExitStack() as top:
        # one packed const tile (tiles pad to 4KB/partition each otherwise):
        # cols 0:16 maskbias, 16:80 ones, 80:208 sel_e, 208:336 sel_o,
        # 336:464 f32-ones row (for K=1 broadcast matmuls)
        const = top.enter_context(tc.tile_pool(name="const", bufs=1))
        cst = const.tile([128, 468], F32, name="cst")
        mask_sb = cst[:, 0:16]
        ones_f = cst[:, 16:80]
        ones_r = cst[:, 336:464]
        eps_c = cst[0:1, 464:465]
        nc.vector.memset(eps_c, EPS)
        nc.sync.dma_start(mask_sb, d_mask.ap())
        nc.vector.memset(cst[:, 16:336], 0.0)
        nc.vector.memset(ones_f, 1.0)
        nc.vector.memset(ones_r, 1.0)
        sel_e = cst[:, 80:208]
        sel_o = cst[:, 208:336]
        nc.vector.memset(sel_e[0:1, 0:64], 1.0)
        nc.vector.memset(sel_e[32:33, 64:128], 1.0)
        nc.vector.memset(sel_o[64:65, 0:64], 1.0)
        nc.vector.memset(sel_o[96:97, 64:128], 1.0)
        ones1_t = const.tile([1, 128], F32R, name="ones1")
        nc.vector.tensor_copy(ones1_t[:], ones_r[0:1, :])
        ones1 = ones1_t[:]                       # [1,128] lhsT for broadcasts
        ones_sb = const.tile([128, 1], DT, name="ones")
        nc.vector.tensor_copy(ones_sb[:], ones_f[:, 0:1])

        p_x1 = top.enter_context(tc.tile_pool(name="px1", bufs=1))
        # first half of fw1 prefetched from kernel start (consumed in stage 4)
        pfw1a = top.enter_context(tc.tile_pool(name="pfw1a", bufs=1))
        fw1a = pfw1a.tile([128, 8, FF // 2], DT, name="fw1a")
        # dedicated early-prefetch pools: post-attention weights + residual
        ppost = top.enter_context(tc.tile_pool(name="ppost", bufs=1))
        wo_sb = ppost.tile([128, 8, D], DT, name="wo")
        aw1_sb = ppost.tile([128, 8, D], DT, name="aw1")
        xq2_sb = ppost.tile([128, 8, SQ], DT, name="xq2")

        def layernorm_block(st, src_sb, gc, bc, res_aps, dst_aps, pref):
            """dst[d] = LN(src)*g + b + res[d]; src [128,8,SQ], dst/res are
            8 per-chunk APs (split tiles let consumers start early)."""
            pln = st.enter_context(tc.tile_pool(name=pref + "ln", bufs=1))
            pps = st.enter_context(tc.tile_pool(name=pref + "lps", bufs=1, space="PSUM"))
            sq_sb = pln.tile([128, 8, SQ], DT, name=pref + "sq")
            for d in range(8):
                nc.vector.tensor_mul(sq_sb[:, d, :], _f(src_sb[:, d, :]),
                                     _f(src_sb[:, d, :]))
            ps_s = pps.tile([1, SQ], F32, name=pref + "ps_s")
            ps_q = pps.tile([1, SQ], F32, name=pref + "ps_q")
            for d in range(8):
                _mm(nc, ps_s[:], ones_sb[:], src_sb[:, d, :],
                    start=(d == 0), stop=(d == 7))
            for d in range(8):
                _mm(nc, ps_q[:], ones_sb[:], sq_sb[:, d, :],
                    start=(d == 0), stop=(d == 7))
            # mu broadcast first: the apply subtracts can start while the
            # variance -> rstd chain is still running
            bc2 = pln.tile([1, 2, SQ], F32R, name=pref + "bc2")
            nc.scalar.mul(bc2[:, 1, :], ps_s[:], 1.0 / D)
            mu = bc2[:, 1, :].bitcast(F32)
            msq = pln.tile([1, SQ], F32, name=pref + "msq")
            nc.scalar.mul(msq[:], ps_q[:], 1.0 / D)
            ps_b = pps.tile([128, 2, SQ], F32, name=pref + "ps_b")
            _mm(nc, ps_b[:, 1, :], ones1, bc2[:, 1, :],
                start=True, stop=True)
            bsb = pln.tile([128, 2, SQ], DT, name=pref + "bsb")
            nc.scalar.copy(bsb[:, 1, :], ps_b[:, 1, :])
            # var = (msq + eps) - mu^2; rstd = sqrt(1/var) via fast DVE
            # approx reciprocal (~4e-6 rel) + the pre-loaded Sqrt table
            mumul = pln.tile([1, SQ], F32, name=pref + "mm")
            nc.vector.tensor_mul(mumul[:], mu, mu)
            var = pln.tile([1, SQ], F32, name=pref + "var")
            nc.vector.scalar_tensor_tensor(var[:], msq[:], EPS, mumul[:],
                                           OP.add, OP.subtract)
            rv = pln.tile([1, SQ], F32, name=pref + "rv")
            nc.vector.reciprocal_approx_fast(out=rv[:], in_=var[:])
            nc.scalar.activation(bc2[:, 0, :], rv[:], AF.Sqrt)
            _mm(nc, ps_b[:, 0, :], ones1, bc2[:, 0, :],
                start=True, stop=True)
            nc.scalar.copy(bsb[:, 0, :], ps_b[:, 0, :])
            tmp = pln.tile([128, 4, SQ], DT, name=pref + "tmp")
            for d in range(8):
                t = tmp[:, d % 4, :]
                nc.vector.tensor_sub(t, _f(src_sb[:, d, :]), _f(bsb[:, 1, :]))
                nc.vector.tensor_mul(t, _f(t), _f(bsb[:, 0, :]))
                if triv:
                    nc.vector.tensor_add(dst_aps[d], _f(t), _f(res_aps[d]))
                else:
                    nc.vector.tensor_scalar(t, _f(t),
                                            gc[:, d:d + 1], bc[:, d:d + 1],
                                            OP.mult, OP.add)
                    nc.vector.tensor_add(dst_aps[d], _f(t), _f(res_aps[d]))

        # ============ Stages 1-3 ============
        with ExitStack() as s13:
            p_acc = s13.enter_context(tc.tile_pool(name="acc", bufs=1))
            acc = p_acc.tile([128, 8, SQ], DT, name="acc")
            # softmax denominators at partition 32*(h%4), free idx h//4;
            # init 1.0 so unused rows stay finite through the reciprocal
            nrm = p_acc.tile([128, 4, SQ], F32, name="nrm")
            nc.vector.memset(nrm[:], 1.0)

            pwkv = s13.enter_context(tc.tile_pool(name="pwkv", bufs=1))
            wk_sb = pwkv.tile([128, 8, D], DT, name="wk")
            wv_sb = pwkv.tile([128, 8, D], DT, name="wv")
            pxsc = s13.enter_context(tc.tile_pool(name="pxsc", bufs=2))

            with ExitStack() as A:
                p_qT = A.enter_context(tc.tile_pool(name="qT", bufs=1))
                qT = p_qT.tile([128, 8, SQ], DT, name="qT")

                # ---- Stage 1a: Q^T projection ----
                with ExitStack() as st:
                    pw = st.enter_context(tc.tile_pool(name="pwq", bufs=1))
                    px = st.enter_context(tc.tile_pool(name="pxq", bufs=1))
                    pp = st.enter_context(tc.tile_pool(name="ppq", bufs=2, space="PSUM"))
                    wq_sb = pw.tile([128, 8, D], DT, name="wq")
                    xq_sb = px.tile([128, 8, SQ], DT, name="xqp")
                    for d in range(8):
                        nc.sync.dma_start(wq_sb[:, d, :], r_wq[:, d, :])
                        nc.sync.dma_start(xq_sb[:, d, :], r_xq[:, d, :])
                    for p in range(8):
                        ps = pp.tile([128, SQ], F32, name="psq")
                        for d in range(8):
                            _mm(nc, ps[:], wq_sb[:, d, p * 128:(p + 1) * 128],
                                xq_sb[:, d, :], start=(d == 0), stop=(d == 7))
                        nc.scalar.copy(qT[:, p, :], ps[:])

                # K/V weights are first needed after Q-proj: issue their DMAs
                # behind the Q-proj inputs so the PE can start ~10us earlier
                for d in range(8):
                    nc.sync.dma_start(wk_sb[:, d, :], r_wk[:, d, :])
                for d in range(8):
                    nc.sync.dma_start(wv_sb[:, d, :], r_wv[:, d, :])

                # pre-issue the first two attention x-chunk DMAs so they beat
                # the big prefetch burst below in scheduler priority order
                xs_tiles = {}

                def issue_xs(sc):
                    t = pxsc.tile([128, 8, 512], DT, name="xsc", tag="xsc")
                    nt = SCG[sc] * 128
                    for d in range(8):
                        nc.sync.dma_start(t[:, d, 0:nt],
                                          r_xT[:, d, sc * 512:sc * 512 + nt])
                    return t

                xs_tiles[0] = issue_xs(0)
                if len(SCG) > 1:
                    xs_tiles[1] = issue_xs(1)

                # prefetch burst: post-attention weights + residual + fw1 half
                # (issued after everything attention-critical)
                for do in range(8):
                    nc.sync.dma_start(wo_sb[:, :, do * 128:(do + 1) * 128],
                                      r_wo[:, :, do * 128:(do + 1) * 128])
                    nc.sync.dma_start(aw1_sb[:, :, do * 128:(do + 1) * 128],
                                      r_aw1[:, :, do * 128:(do + 1) * 128])
                for d in range(8):
                    nc.sync.dma_start(xq2_sb[:, d, :], r_xq[:, d, :])
                for fh in range(4):
                    nc.sync.dma_start(
                        fw1a[:, :, fh * 512:(fh + 1) * 512],
                        r_fw1[:, :, fh * 512:(fh + 1) * 512])

                # ---- Stage 1b+2: K/V proj + attention, flash over scg ----
                pkv = A.enter_context(tc.tile_pool(name="pkv", bufs=2))
                pexp = A.enter_context(tc.tile_pool(name="pexp", bufs=4))
                aps = A.enter_context(ExitStack())
                psc = aps.enter_context(tc.tile_pool(name="psc", bufs=2, space="PSUM"))
                # dedicated K/V-projection psum: keeps next-chunk projection
                # matmuls runnable while scores/PV own the psc/ppv banks
                pkvp = aps.enter_context(tc.tile_pool(name="pkvp", bufs=2, space="PSUM"))
                ppv = aps.enter_context(tc.tile_pool(name="ppv", bufs=1, space="PSUM"))

                for sc, ntc in enumerate(SCG):
                    nt = ntc * 128
                    xs = xs_tiles.pop(sc)
                    if sc + 2 < len(SCG):
                        xs_tiles[sc + 2] = issue_xs(sc + 2)

                    kT = pkv.tile([128, 8, 512], DT, name="kT")
                    for p in range(8):
                        ps = pkvp.tile([128, SQ], F32, name="kvps")
                        for d in range(8):
                            _mm(nc, ps[:, 0:nt], wk_sb[:, d, p * 128:(p + 1) * 128],
                                xs[:, d, 0:nt], start=(d == 0), stop=(d == 7))
                        nc.vector.tensor_copy(kT[:, p, 0:nt], ps[:, 0:nt])

                    vt = pkv.tile([128, 4, 16, 65], DT, name="vt")
                    nc.vector.tensor_copy(
                        vt[:, 0:ntc, :, 64:65],
                        ones_f[:, 0:ntc * 16].rearrange(
                            "p (a b c) -> p a b c", a=ntc, b=16))
                    for i in range(ntc):
                        for nb in range(2):
                            ps = pkvp.tile([128, SQ], F32, name="kvps")
                            for d in range(8):
                                _mm(nc, ps[:], xs[:, d, i * 128:(i + 1) * 128],
                                    wv_sb[:, d, nb * 512:(nb + 1) * 512],
                                    start=(d == 0), stop=(d == 7))
                            nc.vector.tensor_copy(
                                vt[:, i, nb * 8:(nb + 1) * 8, 0:64],
                                ps[:].rearrange("p (h e) -> p h e", e=64))

                    for p in range(8):
                        h0, h1 = 2 * p, 2 * p + 1
                        pva = ppv.tile([128, 2, SQ], F32, name="pva")
                        for i in range(ntc):
                            tci = sc * 4 + i
                            s01 = psc.tile([128, 2, SQ], F32, name="s01")
                            _mm(nc, s01[:, 0, :],
                                kT[0:64, p, i * 128:(i + 1) * 128],
                                qT[0:64, p, :], start=True, stop=True,
                                tile_position=(0, 0))
                            _mm(nc, s01[:, 1, :],
                                kT[64:128, p, i * 128:(i + 1) * 128],
                                qT[64:128, p, :], start=True, stop=True,
                                tile_position=(64, 0))
                            e01 = pexp.tile([128, 2, SQ], DT, name="e01")
                            nc.scalar.activation(e01[:], s01[:], AF.Exp,
                                                 bias=mask_sb[:, tci:tci + 1],
                                                 scale=0.125)
                            _mm(nc, pva[0:65, 0, :], vt[:, i, h0, :], e01[:, 0, :],
                                start=(i == 0), stop=(i == ntc - 1))
                            _mm(nc, pva[0:65, 1, :], vt[:, i, h1, :], e01[:, 1, :],
                                start=(i == 0), stop=(i == ntc - 1))
                        a0, c0 = 32 * (h0 % 4), h0 // 4
                        a1, c1 = 32 * (h1 % 4), h1 // 4
                        if sc == 0:
                            nc.vector.tensor_copy(acc[0:64, p, :], pva[0:64, 0, :])
                            nc.vector.tensor_copy(acc[64:128, p, :], pva[0:64, 1, :])
                            nc.vector.tensor_copy(nrm[a0:a0 + 1, c0, :], pva[64:65, 0, :])
                            nc.vector.tensor_copy(nrm[a1:a1 + 1, c1, :], pva[64:65, 1, :])
                        else:
                            nc.vector.tensor_add(acc[0:64, p, :],
                                                 _f(acc[0:64, p, :]), pva[0:64, 0, :])
                            nc.vector.tensor_add(acc[64:128, p, :],
                                                 _f(acc[64:128, p, :]), pva[0:64, 1, :])
                            nc.vector.tensor_add(nrm[a0:a0 + 1, c0, :],
                                                 nrm[a0:a0 + 1, c0, :], pva[64:65, 0, :])
                            nc.vector.tensor_add(nrm[a1:a1 + 1, c1, :],
                                                 nrm[a1:a1 + 1, c1, :], pva[64:65, 1, :])

                # normalize: acc[:, p, :] *= 1/nrm via selector-matmul bcast;
                # reciprocal per c-group so it pipelines with the tail PVs
                aps.close()
                ppb = A.enter_context(tc.tile_pool(name="ppb", bufs=2, space="PSUM"))
                pnr = A.enter_context(tc.tile_pool(name="pnr", bufs=2))
                for c in range(4):
                    nc.vector.reciprocal_approx_fast(
                        out=nrm[:, c, :], in_=nrm[:, c, :])
                # pre-load the Sqrt ACT table set now (all attention exps are
                # done); keyed off the last reciprocal so the scheduler
                # cannot hoist the table swap above the exps
                scr = pnr.tile([1, 1], F32, name="scr")
                nc.scalar.activation(scr[:], nrm[0:1, 3, 0:1], AF.Sqrt)
                for p in range(8):
                    sel = sel_e if p % 2 == 0 else sel_o
                    ps_rb = ppb.tile([128, SQ], F32, name="ps_rb")
                    nc.tensor.matmul(ps_rb[:], sel, nrm[:, p // 2, :],
                                     start=True, stop=True)
                    nc.vector.tensor_mul(acc[:, p, :], _f(acc[:, p, :]), ps_rb[:])

            # ---- Stage 3: Wo + add1 + LN1 + residual ----
            with ExitStack() as st:
                pw = st.enter_context(tc.tile_pool(name="pw3", bufs=1))
                b1_sb = pw.tile([128, 8], F32, name="b1")
                g1_sb = pw.tile([128, 8], F32, name="g1")
                bb1_sb = pw.tile([128, 8], F32, name="bb1")
                if not triv:
                    nc.sync.dma_start(b1_sb[:], d_b1.ap())
                    nc.sync.dma_start(g1_sb[:], d_g1.ap())
                    nc.sync.dma_start(bb1_sb[:], d_bb1.ap())

                # x1 split into two tiles so FFN1 can start on the first half
                # while the LN apply chain still writes the second
                x1a = p_x1.tile([128, 4, SQ], DT, name="x1a")
                x1b = p_x1.tile([128, 4, SQ], DT, name="x1b")
                x1ap = [x1a[:, d, :] for d in range(4)] + \
                       [x1b[:, d, :] for d in range(4)]
                pao = st.enter_context(tc.tile_pool(name="pao", bufs=1))
                ao = pao.tile([128, 8, SQ], DT, name="ao")
                padd = st.enter_context(tc.tile_pool(name="padd1", bufs=1, space="PSUM"))
                with ExitStack() as stW:
                    pp = stW.enter_context(tc.tile_pool(name="pp3", bufs=2, space="PSUM"))
                    for do in range(8):
                        ps = pp.tile([128, SQ], F32, name="ps3a")
                        for d in range(8):
                            _mm(nc, ps[:], wo_sb[:, d, do * 128:(do + 1) * 128],
                                acc[:, d, :], start=(d == 0), stop=(d == 7))
                        nc.scalar.copy(ao[:, do, :], ps[:])
                # add1 in two 4-bank d-outer passes: pass MMs interleave with
                # the Wo do-loop above as its ao chunks land
                l1 = pao.tile([128, 8, SQ], DT, name="l1")
                for half in range(2):
                    aps4 = padd.tile([128, 4, SQ], F32, name="a1ps", tag="a1ps")
                    for d in range(8):
                        for j in range(4):
                            do = half * 4 + j
                            _mm(nc, aps4[:, j, :],
                                aw1_sb[:, d, do * 128:(do + 1) * 128],
                                ao[:, d, :], start=(d == 0), stop=(d == 7))
                    for j in range(4):
                        do = half * 4 + j
                        if triv:
                            eng = nc.scalar if j < 2 else nc.vector
                            if j < 2:
                                eng.copy(l1[:, do, :], aps4[:, j, :])
                            else:
                                eng.tensor_copy(l1[:, do, :], aps4[:, j, :])
                        else:
                            nc.vector.tensor_scalar(l1[:, do, :], aps4[:, j, :],
                                                    b1_sb[:, do:do + 1], None, OP.add)
                layernorm_block(st, l1, g1_sb, bb1_sb,
                                [xq2_sb[:, d, :] for d in range(8)], x1ap, "a")

        # ================= Stage 4: FFN + add2 + LN2 + residual =================
        with ExitStack() as st:
            pff = st.enter_context(tc.tile_pool(name="pff", bufs=1))
            ff = pff.tile([128, 8, SQ], DT, name="ff")
            aw2_sb = pff.tile([128, 8, D], DT, name="aw2")
            for do in range(8):
                nc.sync.dma_start(aw2_sb[:, :, do * 128:(do + 1) * 128],
                                  r_aw2[:, :, do * 128:(do + 1) * 128])
            padd2 = st.enter_context(tc.tile_pool(name="padd2", bufs=1, space="PSUM"))
            with ExitStack() as st4a:
                ph = st4a.enter_context(tc.tile_pool(name="ph", bufs=1))
                h_sb = ph.tile([128, 32, SQ], DT, name="h")
                pwc = st4a.enter_context(tc.tile_pool(name="pwc", bufs=6))
                pwc2 = st4a.enter_context(tc.tile_pool(name="pwc2", bufs=4))
                fb1_sb = ph.tile([128, 32], F32, name="fb1")
                fb2_sb = ph.tile([128, 8], F32, name="fb2")
                if not triv:
                    nc.sync.dma_start(fb1_sb[:], d_fb1.ap())
                    nc.sync.dma_start(fb2_sb[:], d_fb2.ap())
                pp = st4a.enter_context(tc.tile_pool(name="pp4", bufs=2, space="PSUM"))

                for f in range(32):
                    if f < 16:
                        w1t = fw1a[:, :, f * 128:(f + 1) * 128]
                    else:
                        w1c = pwc.tile([128, 8, 128], DT, name="w1c")
                        nc.sync.dma_start(w1c[:], r_fw1[:, :, f * 128:(f + 1) * 128])
                        w1t = w1c[:]
                    ps = pp.tile([128, SQ], F32, name="ps4a")
                    for d in range(8):
                        _mm(nc, ps[:], w1t[:, d, :], x1ap[d],
                            start=(d == 0), stop=(d == 7))
                    if triv:
                        nc.vector.tensor_scalar(h_sb[:, f, :], ps[:],
                                                0.0, None, OP.max)
                    else:
                        nc.vector.tensor_scalar(h_sb[:, f, :], ps[:],
                                                fb1_sb[:, f:f + 1], 0.0,
                                                OP.add, OP.max)

                for do in range(8):
                    w2t = pwc2.tile([128, 32, 128], DT, name="w2c")
                    nc.sync.dma_start(w2t[:], r_fw2[:, :, do * 128:(do + 1) * 128])
                    ps = pp.tile([128, SQ], F32, name="ps4b")
                    for f in range(32):
                        _mm(nc, ps[:], w2t[:, f, :], h_sb[:, f, :],
                            start=(f == 0), stop=(f == 31))
                    if triv:
                        nc.scalar.copy(ff[:, do, :], ps[:])
                    else:
                        nc.vector.tensor_scalar(ff[:, do, :], ps[:],
                                                fb2_sb[:, do:do + 1], None, OP.add)

            with ExitStack() as st4b:
                pw = st4b.enter_context(tc.tile_pool(name="pw4", bufs=1))
                b2_sb = pw.tile([128, 8], F32, name="b2")
                g2_sb = pw.tile([128, 8], F32, name="g2")
                bb2_sb = pw.tile([128, 8], F32, name="bb2")
                if not triv:
                    nc.sync.dma_start(b2_sb[:], d_b2.ap())
                    nc.sync.dma_start(g2_sb[:], d_g2.ap())
                    nc.sync.dma_start(bb2_sb[:], d_bb2.ap())

                # add2 in two 4-bank d-outer passes: pass-A MMs interleave
                # with the FFN2 do-loop as its ff chunks land
                l2 = pw.tile([128, 8, SQ], DT, name="l2")
                for half in range(2):
                    aps4 = padd2.tile([128, 4, SQ], F32, name="a2ps", tag="a2ps")
                    for d in range(8):
                        for j in range(4):
                            do = half * 4 + j
                            _mm(nc, aps4[:, j, :],
                                aw2_sb[:, d, do * 128:(do + 1) * 128],
                                ff[:, d, :], start=(d == 0), stop=(d == 7))
                    for j in range(4):
                        do = half * 4 + j
                        if triv:
                            if j < 2:
                                nc.scalar.copy(l2[:, do, :], aps4[:, j, :])
                            else:
                                nc.vector.tensor_copy(l2[:, do, :], aps4[:, j, :])
                        else:
                            nc.vector.tensor_scalar(l2[:, do, :], aps4[:, j, :],
                                                    b2_sb[:, do:do + 1], None, OP.add)

                outa = pw.tile([128, 4, SQ], DT, name="outa")
                outb = pw.tile([128, 4, SQ], DT, name="outb")
                outap = [outa[:, d, :] for d in range(4)] + \
                        [outb[:, d, :] for d in range(4)]
                layernorm_block(st4b, l2, g2_sb, bb2_sb, x1ap, outap, "b")
                for d in range(8):
                    nc.sync.dma_start(r_out[:, d, :], outap[d])

    nc.compile()
    return nc


_NC = {}


def _get_nc(nact, triv):
    key = (nact, triv, _MODE)
    if key not in _NC:
        _NC[key] = build_nc(nact, triv)
    return _NC[key]


def _prep_inputs(inputs):
    """Host-side shard prep: per-core input dicts."""
    x = np.asarray(inputs["batch_x"], np.float32)       # [B, S, D]
    lens = np.asarray(inputs["len_chair"], np.int64)
    wq = np.ascontiguousarray(
        np.asarray(inputs["Wq"], np.float32).transpose(1, 0, 2).reshape(D, D))
    wk = np.ascontiguousarray(
        np.asarray(inputs["Wk"], np.float32).transpose(1, 0, 2).reshape(D, D))
    wv = np.ascontiguousarray(
        np.asarray(inputs["Wv"], np.float32).transpose(1, 0, 2).reshape(D, D))
    com = {
        "wq": wq, "wk": wk, "wv": wv,
        "wo": np.ascontiguousarray(np.asarray(inputs["Wo"], np.float32)),
        "aw1": np.ascontiguousarray(np.asarray(inputs["add1_w"], np.float32)),
        "aw2": np.ascontiguousarray(np.asarray(inputs["add2_w"], np.float32)),
        "fw1": np.ascontiguousarray(np.asarray(inputs["ff_w1"], np.float32)),
        "fw2": np.ascontiguousarray(np.asarray(inputs["ff_w2"], np.float32)),
        "b1c": _chunk(inputs["add1_b"]), "g1c": _chunk(inputs["ln1_g"]),
        "bb1c": _chunk(inputs["ln1_b"]), "fb1c": _chunk(inputs["ff_b1"]),
        "fb2c": _chunk(inputs["ff_b2"]), "b2c": _chunk(inputs["add2_b"]),
        "g2c": _chunk(inputs["ln2_g"]), "bb2c": _chunk(inputs["ln2_b"]),
    }
    xT = [np.ascontiguousarray(x[b].T) for b in range(B)]   # [D, S]
    masks = []
    for b in range(B):
        m = np.where(np.arange(S) >= lens[b], np.float32(MASK_NEG),
                     np.float32(0.0)).astype(np.float32)
        masks.append(np.ascontiguousarray(m.reshape(S // 128, 128).T))
    in_maps = []
    for c in range(NCORES):
        b, q = c // 4, c % 4
        m = dict(com)
        m["xT"] = xT[b]
        m["xq"] = np.ascontiguousarray(xT[b][:, q * SQ:(q + 1) * SQ])
        m["maskb"] = masks[b]
        in_maps.append(m)
    return in_maps


def _chunk(v):
    v = np.asarray(v, np.float32)
    return np.ascontiguousarray(v.reshape(-1, 128).T)


DT_KEYS = ("xT", "xq", "wq", "wk", "wv", "wo", "aw1", "aw2", "fw1", "fw2")


def kernel(trace=False, **inputs):
    _install_ntff_hook()
    lens = np.asarray(inputs["len_chair"], np.int64)
    nact = int(max(1, min(S // 128, -(-int(lens.max()) // 128))))
    triv = (np.all(np.asarray(inputs["ln1_g"]) == 1.0)
            and np.all(np.asarray(inputs["ln2_g"]) == 1.0)
            and not np.any(np.asarray(inputs["ln1_b"]))
            and not np.any(np.asarray(inputs["ln2_b"]))
            and not np.any(np.asarray(inputs["add1_b"]))
            and not np.any(np.asarray(inputs["add2_b"]))
            and not np.any(np.asarray(inputs["ff_b1"]))
            and not np.any(np.asarray(inputs["ff_b2"])))
    nc = _get_nc(nact, triv)
    in_maps = _prep_inputs(inputs)
    import ml_dtypes
    np_dt = mybir.dt.np(DT)
    cache = {}

    def _cast(a, dtype):
        key = (id(a), np.dtype(dtype).str)
        if key not in cache:
            cache[key] = np.ascontiguousarray(a.astype(dtype))
        return cache[key]

    for m in in_maps:
        if np_dt != np.float32:
            for k in DT_KEYS:
                m[k] = _cast(m[k], np_dt)
    res = run_bass_kernel_spmd(nc, in_maps, core_ids=list(range(NCORES)),
                               trace=trace)
    out = np.empty((B, S, D), np.float32)
    for c in range(NCORES):
        b, q = c // 4, c % 4
        out[b, q * SQ:(q + 1) * SQ, :] = \
            np.asarray(res.results[c]["out"]).astype(np.float32).T
    kernel.last_exec_time_ns = res.exec_time_ns
    return out


# revision 49
# speedup vs baseline: 1.0359x; 1.0359x over previous
"""Trainium2 Bass kernel for nn_Encoder_Block (B=2,S=2048,D=1024,H=16,FF=4096).

Sharding: 8 cores, core c -> (batch b=c//4, query block q=c%4 of 512 tokens).
Each core recomputes K/V for its whole batch (no cross-core collectives),
everything else is perfectly sharded. Host does transposes and gather.

Device layout: activations kept transposed [feature, token] throughout, so
every matmul in the chain is a natural lhsT/rhs pair with K=128 contraction
chunks and N=512 moving dim. Attention computes transposed scores [t, sq];
softmax normalizer rides along the PV matmul as a ones-column in V (M=65).
Masking + 1/sqrt(dh) scaling are folded into the Exp activation (bias/scale).
No max-subtraction: scores are O(1) by construction, exp is safe in fp32.

v2: bf16 matmul path, fully-masked key chunks skipped (program specialized
on ceil(max(len)/128) at runtime), dedicated early weight prefetch, fast
approx reciprocal, trivial-LN fast path (g==1, b==0 checked host-side),
broadcasts done as K=1 matmuls on the (otherwise idle) PE.
"""
import sys, types, os
sys.path.insert(0, "/opt/trn_rl_repo")
import numpy as np
from contextlib import ExitStack

import concourse.bass as bass
import concourse.tile as tile
from concourse import bacc, mybir
from concourse.bass_utils import run_bass_kernel_spmd

B, S, D, H, FF = 2, 2048, 1024, 16, 4096
DH = D // H            # 64
SQ = 512               # query tokens per core
NCORES = 8
EPS = 1e-5
MASK_NEG = -60.0       # exp(-60) ~ 8.8e-27 => masked keys contribute ~0

F32 = mybir.dt.float32
F32R = mybir.dt.float32r
# PE compute dtype for matmul-fed tensors:
#   bf16 : 1 cyc/row, half DMA/LDWEIGHTS cost, ~5e-3 output error
#   f32r : 1 cyc/row TF32-like, ~3.5e-4 output error
_MODE = os.environ.get("KERNEL_MM_DT", "bf16")
DT = {"f32r": mybir.dt.float32r, "f32": mybir.dt.float32,
      "bf16": mybir.dt.bfloat16}[_MODE]


def _f(ap):
    # f32r tiles must be bitcast to f32 for DVE/ACT reads; bf16 is native
    if DT == mybir.dt.float32r:
        return ap.bitcast(F32)
    return ap


def _install_ntff_hook():
    """The image's antenv lacks axon_hooks; shim it so trace=True works."""
    try:
        import antenv.axon_hooks  # noqa
        return
    except ImportError:
        pass
    try:
        from trn_agent_boot.trn_boot import _ntff_profile_via_ctypes
        import antenv
        mod = types.ModuleType("antenv.axon_hooks")
        hook = _ntff_profile_via_ctypes("/opt/axon/libaxon_pjrt.so")
        mod.get_axon_ntff_profile_hook = lambda: hook
        mod.set_axon_ntff_profile_hook = lambda h: None
        sys.modules["antenv.axon_hooks"] = mod
        antenv.axon_hooks = mod
    except Exception:
        pass


def _mm(nc, out, lhsT, rhs, start, stop, tile_position=None):
    nc.tensor.matmul(out, lhsT, rhs,
                     start=start, stop=stop, tile_position=tile_position)


def build_nc(nact=16, triv=True):
    """nact: number of active 128-key chunks (fully-masked tail skipped).
    triv: all LN gammas are 1, all betas/biases 0 (checked host-side)."""
    nc = bacc.Bacc(trn_type="TRN2", target_bir_lowering=False, debug=False,
                   num_devices=NCORES, dynamic_dma_scratch_size=512)
    AF = mybir.ActivationFunctionType
    OP = mybir.AluOpType

    # sc groups of up to 4 active 128-token chunks each
    SCG = []
    rem = nact
    while rem > 0:
        SCG.append(min(4, rem))
        rem -= 4

    # ---- DRAM I/O (per-core; program identical across cores) ----
    d_xT = nc.dram_tensor("xT", [D, S], DT, kind="ExternalInput")
    d_xq = nc.dram_tensor("xq", [D, SQ], DT, kind="ExternalInput")
    d_mask = nc.dram_tensor("maskb", [128, S // 128], F32, kind="ExternalInput")
    d_wq = nc.dram_tensor("wq", [D, D], DT, kind="ExternalInput")
    d_wk = nc.dram_tensor("wk", [D, D], DT, kind="ExternalInput")
    d_wv = nc.dram_tensor("wv", [D, D], DT, kind="ExternalInput")
    d_wo = nc.dram_tensor("wo", [D, D], DT, kind="ExternalInput")
    d_aw1 = nc.dram_tensor("aw1", [D, D], DT, kind="ExternalInput")
    d_aw2 = nc.dram_tensor("aw2", [D, D], DT, kind="ExternalInput")
    d_fw1 = nc.dram_tensor("fw1", [D, FF], DT, kind="ExternalInput")
    d_fw2 = nc.dram_tensor("fw2", [FF, D], DT, kind="ExternalInput")
    d_b1 = nc.dram_tensor("b1c", [128, 8], F32, kind="ExternalInput")
    d_g1 = nc.dram_tensor("g1c", [128, 8], F32, kind="ExternalInput")
    d_bb1 = nc.dram_tensor("bb1c", [128, 8], F32, kind="ExternalInput")
    d_fb1 = nc.dram_tensor("fb1c", [128, 32], F32, kind="ExternalInput")
    d_fb2 = nc.dram_tensor("fb2c", [128, 8], F32, kind="ExternalInput")
    d_b2 = nc.dram_tensor("b2c", [128, 8], F32, kind="ExternalInput")
    d_g2 = nc.dram_tensor("g2c", [128, 8], F32, kind="ExternalInput")
    d_bb2 = nc.dram_tensor("bb2c", [128, 8], F32, kind="ExternalInput")
    d_out = nc.dram_tensor("out", [D, SQ], DT, kind="ExternalOutput")

    r_xT = d_xT.ap().rearrange("(c p) s -> p c s", p=128)     # [128, 8, S]
    r_xq = d_xq.ap().rearrange("(c p) s -> p c s", p=128)     # [128, 8, SQ]
    r_wq = d_wq.ap().rearrange("(c p) n -> p c n", p=128)
    r_wk = d_wk.ap().rearrange("(c p) n -> p c n", p=128)
    r_wv = d_wv.ap().rearrange("(c p) n -> p c n", p=128)
    r_wo = d_wo.ap().rearrange("(c p) n -> p c n", p=128)
    r_aw1 = d_aw1.ap().rearrange("(c p) n -> p c n", p=128)
    r_aw2 = d_aw2.ap().rearrange("(c p) n -> p c n", p=128)
    r_fw1 = d_fw1.ap().rearrange("(c p) n -> p c n", p=128)   # [128, 8, FF]
    r_fw2 = d_fw2.ap().rearrange("(c p) n -> p c n", p=128)   # [128, 32, D]
    r_out = d_out.ap().rearrange("(c p) s -> p c s", p=128)

    with tile.TileContext(nc) as tc:
      with ExitStack() as top:
        # one packed const tile (tiles pad to 4KB/partition each otherwise):
        # cols 0:16 maskbias, 16:80 ones, 80:208 sel_e, 208:336 sel_o,
        # 336:464 f32-ones row (for K=1 broadcast matmuls)
        const = top.enter_context(tc.tile_pool(name="const", bufs=1))
        cst = const.tile([128, 468], F32, name="cst")
        mask_sb = cst[:, 0:16]
        ones_f = cst[:, 16:80]
        ones_r = cst[:, 336:464]
        eps_c = cst[0:1, 464:465]
        nc.vector.memset(eps_c, EPS)
        nc.sync.dma_start(mask_sb, d_mask.ap())
        nc.vector.memset(cst[:, 16:336], 0.0)
        nc.vector.memset(ones_f, 1.0)
        nc.vector.memset(ones_r, 1.0)
        sel_e = cst[:, 80:208]
        sel_o = cst[:, 208:336]
        nc.vector.memset(sel_e[0:1, 0:64], 1.0)
        nc.vector.memset(sel_e[32:33, 64:128], 1.0)
        nc.vector.memset(sel_o[64:65, 0:64], 1.0)
        nc.vector.memset(sel_o[96:97, 64:128], 1.0)
        ones1_t = const.tile([1, 128], F32R, name="ones1")
        nc.vector.tensor_copy(ones1_t[:], ones_r[0:1, :])
        ones1 = ones1_t[:]                       # [1,128] lhsT for broadcasts
        ones_sb = const.tile([128, 1], DT, name="ones")
        nc.vector.tensor_copy(ones_sb[:], ones_f[:, 0:1])

        p_x1 = top.enter_context(tc.tile_pool(name="px1", bufs=1))
        # first half of fw1 prefetched from kernel start (consumed in stage 4)
        pfw1a = top.enter_context(tc.tile_pool(name="pfw1a", bufs=1))
        fw1a = pfw1a.tile([128, 8, FF // 2], DT, name="fw1a")
        # dedicated early-prefetch pools: post-attention weights + residual
        ppost = top.enter_context(tc.tile_pool(name="ppost", bufs=1))
        wo_sb = ppost.tile([128, 8, D], DT, name="wo")
        aw1_sb = ppost.tile([128, 8, D], DT, name="aw1")
        xq2_sb = ppost.tile([128, 8, SQ], DT, name="xq2")

        def layernorm_block(st, src_sb, gc, bc, res_aps, dst_aps, pref):
            """dst[d] = LN(src)*g + b + res[d]; src [128,8,SQ], dst/res are
            8 per-chunk APs (split tiles let consumers start early)."""
            pln = st.enter_context(tc.tile_pool(name=pref + "ln", bufs=1))
            pps = st.enter_context(tc.tile_pool(name=pref + "lps", bufs=1, space="PSUM"))
            sq_sb = pln.tile([128, 8, SQ], DT, name=pref + "sq")
            for d in range(8):
                nc.vector.tensor_mul(sq_sb[:, d, :], _f(src_sb[:, d, :]),
                                     _f(src_sb[:, d, :]))
            ps_s = pps.tile([1, SQ], F32, name=pref + "ps_s")
            ps_q = pps.tile([1, SQ], F32, name=pref + "ps_q")
            for d in range(8):
                _mm(nc, ps_s[:], ones_sb[:], src_sb[:, d, :],
                    start=(d == 0), stop=(d == 7))
            for d in range(8):
                _mm(nc, ps_q[:], ones_sb[:], sq_sb[:, d, :],
                    start=(d == 0), stop=(d == 7))
            # mu broadcast first: the apply subtracts can start while the
            # variance -> rstd chain is still running
            bc2 = pln.tile([1, 2, SQ], F32R, name=pref + "bc2")
            nc.scalar.mul(bc2[:, 1, :], ps_s[:], 1.0 / D)
            mu = bc2[:, 1, :].bitcast(F32)
            msq = pln.tile([1, SQ], F32, name=pref + "msq")
            nc.scalar.mul(msq[:], ps_q[:], 1.0 / D)
            ps_b = pps.tile([128, 2, SQ], F32, name=pref + "ps_b")
            _mm(nc, ps_b[:, 1, :], ones1, bc2[:, 1, :],
                start=True, stop=True)
            bsb = pln.tile([128, 2, SQ], DT, name=pref + "bsb")
            nc.scalar.copy(bsb[:, 1, :], ps_b[:, 1, :])
            # var = (msq + eps) - mu^2; rstd = sqrt(1/var) via fast DVE
            # approx reciprocal (~4e-6 rel) + the pre-loaded Sqrt table
            mumul = pln.tile([1, SQ], F32, name=pref + "mm")
            nc.vector.tensor_mul(mumul[:], mu, mu)
            var = pln.tile([1, SQ], F32, name=pref + "var")
            nc.vector.scalar_tensor_tensor(var[:], msq[:], EPS, mumul[:],
                                           OP.add, OP.subtract)
            rv = pln.tile([1, SQ], F32, name=pref + "rv")
            nc.vector.reciprocal_approx_fast(out=rv[:], in_=var[:])
            nc.scalar.activation(bc2[:, 0, :], rv[:], AF.Sqrt)
            _mm(nc, ps_b[:, 0, :], ones1, bc2[:, 0, :],
                start=True, stop=True)
            nc.scalar.copy(bsb[:, 0, :], ps_b[:, 0, :])
            tmp = pln.tile([128, 4, SQ], DT, name=pref + "tmp")
            for d in range(8):
                t = tmp[:, d % 4, :]
                nc.vector.tensor_sub(t, _f(src_sb[:, d, :]), _f(bsb[:, 1, :]))
                nc.vector.tensor_mul(t, _f(t), _f(bsb[:, 0, :]))
                if triv:
                    nc.vector.tensor_add(dst_aps[d], _f(t), _f(res_aps[d]))
                else:
                    nc.vector.tensor_scalar(t, _f(t),
                                            gc[:, d:d + 1], bc[:, d:d + 1],
                                            OP.mult, OP.add)
                    nc.vector.tensor_add(dst_aps[d], _f(t), _f(res_aps[d]))

        # ============ Stages 1-3 ============
        with ExitStack() as s13:
            p_acc = s13.enter_context(tc.tile_pool(name="acc", bufs=1))
            acc = p_acc.tile([128, 8, SQ], DT, name="acc")
            # softmax denominators at partition 32*(h%4), free idx h//4;
            # init 1.0 so unused rows stay finite through the reciprocal
            nrm = p_acc.tile([128, 4, SQ], F32, name="nrm")
            nc.vector.memset(nrm[:], 1.0)

            pwkv = s13.enter_context(tc.tile_pool(name="pwkv", bufs=1))
            wk_sb = pwkv.tile([128, 8, D], DT, name="wk")
            wv_sb = pwkv.tile([128, 8, D], DT, name="wv")
            pxsc = s13.enter_context(tc.tile_pool(name="pxsc", bufs=2))

            with ExitStack() as A:
                p_qT = A.enter_context(tc.tile_pool(name="qT", bufs=1))
                qT = p_qT.tile([128, 8, SQ], DT, name="qT")

                # ---- Stage 1a: Q^T projection ----
                with ExitStack() as st:
                    pw = st.enter_context(tc.tile_pool(name="pwq", bufs=1))
                    px = st.enter_context(tc.tile_pool(name="pxq", bufs=1))
                    pp = st.enter_context(tc.tile_pool(name="ppq", bufs=2, space="PSUM"))
                    wq_sb = pw.tile([128, 8, D], DT, name="wq")
                    xq_sb = px.tile([128, 8, SQ], DT, name="xqp")
                    for d in range(8):
                        nc.sync.dma_start(wq_sb[:, d, :], r_wq[:, d, :])
                        nc.sync.dma_start(xq_sb[:, d, :], r_xq[:, d, :])
                    for p in range(8):
                        ps = pp.tile([128, SQ], F32, name="psq")
                        for d in range(8):
                            _mm(nc, ps[:], wq_sb[:, d, p * 128:(p + 1) * 128],
                                xq_sb[:, d, :], start=(d == 0), stop=(d == 7))
                        nc.scalar.copy(qT[:, p, :], ps[:])

                # K/V weights are first needed after Q-proj: issue their DMAs
                # behind the Q-proj inputs so the PE can start ~10us earlier
                for d in range(8):
                    nc.sync.dma_start(wk_sb[:, d, :], r_wk[:, d, :])
                for d in range(8):
                    nc.sync.dma_start(wv_sb[:, d, :], r_wv[:, d, :])

                # pre-issue the first two attention x-chunk DMAs so they beat
                # the big prefetch burst below in scheduler priority order
                xs_tiles = {}

                def issue_xs(sc):
                    t = pxsc.tile([128, 8, 512], DT, name="xsc", tag="xsc")
                    nt = SCG[sc] * 128
                    for d in range(8):
                        nc.sync.dma_start(t[:, d, 0:nt],
                                          r_xT[:, d, sc * 512:sc * 512 + nt])
                    return t

                xs_tiles[0] = issue_xs(0)
                if len(SCG) > 1:
                    xs_tiles[1] = issue_xs(1)

                # prefetch burst: post-attention weights + residual + fw1 half
                # (issued after everything attention-critical)
                for do in range(8):
                    nc.sync.dma_start(wo_sb[:, :, do * 128:(do + 1) * 128],
                                      r_wo[:, :, do * 128:(do + 1) * 128])
                    nc.sync.dma_start(aw1_sb[:, :, do * 128:(do + 1) * 128],
                                      r_aw1[:, :, do * 128:(do + 1) * 128])
                for d in range(8):
                    nc.sync.dma_start(xq2_sb[:, d, :], r_xq[:, d, :])
                for fh in range(4):
                    nc.sync.dma_start(
                        fw1a[:, :, fh * 512:(fh + 1) * 512],
                        r_fw1[:, :, fh * 512:(fh + 1) * 512])

                # ---- Stage 1b+2: K/V proj + attention, flash over scg ----
                pkv = A.enter_context(tc.tile_pool(name="pkv", bufs=2))
                pexp = A.enter_context(tc.tile_pool(name="pexp", bufs=4))
                aps = A.enter_context(ExitStack())
                psc = aps.enter_context(tc.tile_pool(name="psc", bufs=2, space="PSUM"))
                # dedicated K/V-projection psum: keeps next-chunk projection
                # matmuls runnable while scores/PV own the psc/ppv banks
                pkvp = aps.enter_context(tc.tile_pool(name="pkvp", bufs=2, space="PSUM"))
                ppv = aps.enter_context(tc.tile_pool(name="ppv", bufs=1, space="PSUM"))

                for sc, ntc in enumerate(SCG):
                    nt = ntc * 128
                    xs = xs_tiles.pop(sc)
                    if sc + 2 < len(SCG):
                        xs_tiles[sc + 2] = issue_xs(sc + 2)

                    kT = pkv.tile([128, 8, 512], DT, name="kT")
                    for p in range(8):
                        ps = pkvp.tile([128, SQ], F32, name="kvps")
                        for d in range(8):
                            _mm(nc, ps[:, 0:nt], wk_sb[:, d, p * 128:(p + 1) * 128],
                                xs[:, d, 0:nt], start=(d == 0), stop=(d == 7))
                        nc.vector.tensor_copy(kT[:, p, 0:nt], ps[:, 0:nt])

                    vt = pkv.tile([128, 4, 16, 65], DT, name="vt")
                    nc.vector.tensor_copy(
                        vt[:, 0:ntc, :, 64:65],
                        ones_f[:, 0:ntc * 16].rearrange(
                            "p (a b c) -> p a b c", a=ntc, b=16))
                    for i in range(ntc):
                        for nb in range(2):
                            ps = pkvp.tile([128, SQ], F32, name="kvps")
                            for d in range(8):
                                _mm(nc, ps[:], xs[:, d, i * 128:(i + 1) * 128],
                                    wv_sb[:, d, nb * 512:(nb + 1) * 512],
                                    start=(d == 0), stop=(d == 7))
                            nc.vector.tensor_copy(
                                vt[:, i, nb * 8:(nb + 1) * 8, 0:64],
                                ps[:].rearrange("p (h e) -> p h e", e=64))

                    for p in range(8):
                        h0, h1 = 2 * p, 2 * p + 1
                        pva = ppv.tile([128, 2, SQ], F32, name="pva")
                        for i in range(ntc):
                            tci = sc * 4 + i
                            s01 = psc.tile([128, 2, SQ], F32, name="s01")
                            _mm(nc, s01[:, 0, :],
                                kT[0:64, p, i * 128:(i + 1) * 128],
                                qT[0:64, p, :], start=True, stop=True,
                                tile_position=(0, 0))
                            _mm(nc, s01[:, 1, :],
                                kT[64:128, p, i * 128:(i + 1) * 128],
                                qT[64:128, p, :], start=True, stop=True,
                                tile_position=(64, 0))
                            e01 = pexp.tile([128, 2, SQ], DT, name="e01")
                            nc.scalar.activation(e01[:], s01[:], AF.Exp,
                                                 bias=mask_sb[:, tci:tci + 1],
                                                 scale=0.125)
                            _mm(nc, pva[0:65, 0, :], vt[:, i, h0, :], e01[:, 0, :],
                                start=(i == 0), stop=(i == ntc - 1))
                            _mm(nc, pva[0:65, 1, :], vt[:, i, h1, :], e01[:, 1, :],
                                start=(i == 0), stop=(i == ntc - 1))
                        a0, c0 = 32 * (h0 % 4), h0 // 4
                        a1, c1 = 32 * (h1 % 4), h1 // 4
                        if sc == 0:
                            nc.vector.tensor_copy(acc[0:64, p, :], pva[0:64, 0, :])
                            nc.vector.tensor_copy(acc[64:128, p, :], pva[0:64, 1, :])
                            nc.vector.tensor_copy(nrm[a0:a0 + 1, c0, :], pva[64:65, 0, :])
                            nc.vector.tensor_copy(nrm[a1:a1 + 1, c1, :], pva[64:65, 1, :])
                        else:
                            nc.vector.tensor_add(acc[0:64, p, :],
                                                 _f(acc[0:64, p, :]), pva[0:64, 0, :])
                            nc.vector.tensor_add(acc[64:128, p, :],
                                                 _f(acc[64:128, p, :]), pva[0:64, 1, :])
                            nc.vector.tensor_add(nrm[a0:a0 + 1, c0, :],
                                                 nrm[a0:a0 + 1, c0, :], pva[64:65, 0, :])
                            nc.vector.tensor_add(nrm[a1:a1 + 1, c1, :],
                                                 nrm[a1:a1 + 1, c1, :], pva[64:65, 1, :])

                # normalize: acc[:, p, :] *= 1/nrm via selector-matmul bcast;
                # reciprocal per c-group so it pipelines with the tail PVs
                aps.close()
                ppb = A.enter_context(tc.tile_pool(name="ppb", bufs=2, space="PSUM"))
                pnr = A.enter_context(tc.tile_pool(name="pnr", bufs=2))
                for c in range(4):
                    nc.vector.reciprocal_approx_fast(
                        out=nrm[:, c, :], in_=nrm[:, c, :])
                # pre-load the Sqrt ACT table set now (all attention exps are
                # done); keyed off the last reciprocal so the scheduler
                # cannot hoist the table swap above the exps
                scr = pnr.tile([1, 1], F32, name="scr")
                nc.scalar.activation(scr[:], nrm[0:1, 3, 0:1], AF.Sqrt)
                for p in range(8):
                    sel = sel_e if p % 2 == 0 else sel_o
                    ps_rb = ppb.tile([128, SQ], F32, name="ps_rb")
                    nc.tensor.matmul(ps_rb[:], sel, nrm[:, p // 2, :],
                                     start=True, stop=True)
                    nc.vector.tensor_mul(acc[:, p, :], _f(acc[:, p, :]), ps_rb[:])

            # ---- Stage 3: Wo + add1 + LN1 + residual ----
            with ExitStack() as st:
                pw = st.enter_context(tc.tile_pool(name="pw3", bufs=1))
                b1_sb = pw.tile([128, 8], F32, name="b1")
                g1_sb = pw.tile([128, 8], F32, name="g1")
                bb1_sb = pw.tile([128, 8], F32, name="bb1")
                if not triv:
                    nc.sync.dma_start(b1_sb[:], d_b1.ap())
                    nc.sync.dma_start(g1_sb[:], d_g1.ap())
                    nc.sync.dma_start(bb1_sb[:], d_bb1.ap())

                # x1 split into two tiles so FFN1 can start on the first half
                # while the LN apply chain still writes the second
                x1a = p_x1.tile([128, 4, SQ], DT, name="x1a")
                x1b = p_x1.tile([128, 4, SQ], DT, name="x1b")
                x1ap = [x1a[:, d, :] for d in range(4)] + \
                       [x1b[:, d, :] for d in range(4)]
                pao = st.enter_context(tc.tile_pool(name="pao", bufs=1))
                ao = pao.tile([128, 8, SQ], DT, name="ao")
                padd = st.enter_context(tc.tile_pool(name="padd1", bufs=1, space="PSUM"))
                with ExitStack() as stW:
                    pp = stW.enter_context(tc.tile_pool(name="pp3", bufs=2, space="PSUM"))
                    for do in range(8):
                        ps = pp.tile([128, SQ], F32, name="ps3a")
                        for d in range(8):
                            _mm(nc, ps[:], wo_sb[:, d, do * 128:(do + 1) * 128],
                                acc[:, d, :], start=(d == 0), stop=(d == 7))
                        nc.scalar.copy(ao[:, do, :], ps[:])
                # add1 in two 4-bank d-outer passes: pass MMs interleave with
                # the Wo do-loop above as its ao chunks land
                l1 = pao.tile([128, 8, SQ], DT, name="l1")
                for half in range(2):
                    aps4 = padd.tile([128, 4, SQ], F32, name="a1ps", tag="a1ps")
                    for d in range(8):
                        for j in range(4):
                            do = half * 4 + j
                            _mm(nc, aps4[:, j, :],
                                aw1_sb[:, d, do * 128:(do + 1) * 128],
                                ao[:, d, :], start=(d == 0), stop=(d == 7))
                    for j in range(4):
                        do = half * 4 + j
                        if triv:
                            eng = nc.scalar if j < 2 else nc.vector
                            if j < 2:
                                eng.copy(l1[:, do, :], aps4[:, j, :])
                            else:
                                eng.tensor_copy(l1[:, do, :], aps4[:, j, :])
                        else:
                            nc.vector.tensor_scalar(l1[:, do, :], aps4[:, j, :],
                                                    b1_sb[:, do:do + 1], None, OP.add)
                layernorm_block(st, l1, g1_sb, bb1_sb,
                                [xq2_sb[:, d, :] for d in range(8)], x1ap, "a")

        # ================= Stage 4: FFN + add2 + LN2 + residual =================
        with ExitStack() as st:
            pff = st.enter_context(tc.tile_pool(name="pff", bufs=1))
            ff = pff.tile([128, 8, SQ], DT, name="ff")
            aw2_sb = pff.tile([128, 8, D], DT, name="aw2")
            for do in range(8):
                nc.sync.dma_start(aw2_sb[:, :, do * 128:(do + 1) * 128],
                                  r_aw2[:, :, do * 128:(do + 1) * 128])
            padd2 = st.enter_context(tc.tile_pool(name="padd2", bufs=1, space="PSUM"))
            with ExitStack() as st4a:
                ph = st4a.enter_context(tc.tile_pool(name="ph", bufs=1))
                h_sb = ph.tile([128, 32, SQ], DT, name="h")
                pwc = st4a.enter_context(tc.tile_pool(name="pwc", bufs=6))
                pwc2 = st4a.enter_context(tc.tile_pool(name="pwc2", bufs=4))
                fb1_sb = ph.tile([128, 32], F32, name="fb1")
                fb2_sb = ph.tile([128, 8], F32, name="fb2")
                if not triv:
                    nc.sync.dma_start(fb1_sb[:], d_fb1.ap())
                    nc.sync.dma_start(fb2_sb[:], d_fb2.ap())
                pp = st4a.enter_context(tc.tile_pool(name="pp4", bufs=2, space="PSUM"))

                for f in range(32):
                    if f < 16:
                        w1t = fw1a[:, :, f * 128:(f + 1) * 128]
                    else:
                        w1c = pwc.tile([128, 8, 128], DT, name="w1c")
                        nc.sync.dma_start(w1c[:], r_fw1[:, :, f * 128:(f + 1) * 128])
                        w1t = w1c[:]
                    ps = pp.tile([128, SQ], F32, name="ps4a")
                    for d in range(8):
                        _mm(nc, ps[:], w1t[:, d, :], x1ap[d],
                            start=(d == 0), stop=(d == 7))
                    if triv:
                        nc.vector.tensor_scalar(h_sb[:, f, :], ps[:],
                                                0.0, None, OP.max)
                    else:
                        nc.vector.tensor_scalar(h_sb[:, f, :], ps[:],
                                                fb1_sb[:, f:f + 1], 0.0,
                                                OP.add, OP.max)

                for do in range(8):
                    w2t = pwc2.tile([128, 32, 128], DT, name="w2c")
                    nc.sync.dma_start(w2t[:], r_fw2[:, :, do * 128:(do + 1) * 128])
                    ps = pp.tile([128, SQ], F32, name="ps4b")
                    for f in range(32):
                        _mm(nc, ps[:], w2t[:, f, :], h_sb[:, f, :],
                            start=(f == 0), stop=(f == 31))
                    if triv:
                        nc.scalar.copy(ff[:, do, :], ps[:])
                    else:
                        nc.vector.tensor_scalar(ff[:, do, :], ps[:],
                                                fb2_sb[:, do:do + 1], None, OP.add)

            with ExitStack() as st4b:
                pw = st4b.enter_context(tc.tile_pool(name="pw4", bufs=1))
                b2_sb = pw.tile([128, 8], F32, name="b2")
                g2_sb = pw.tile([128, 8], F32, name="g2")
                bb2_sb = pw.tile([128, 8], F32, name="bb2")
                if not triv:
                    nc.sync.dma_start(b2_sb[:], d_b2.ap())
                    nc.sync.dma_start(g2_sb[:], d_g2.ap())
                    nc.sync.dma_start(bb2_sb[:], d_bb2.ap())

                # add2 in two 4-bank d-outer passes: pass-A MMs interleave
                # with the FFN2 do-loop as its ff chunks land
                l2 = pw.tile([128, 8, SQ], DT, name="l2")
                for half in range(2):
                    aps4 = padd2.tile([128, 4, SQ], F32, name="a2ps", tag="a2ps")
                    for d in range(8):
                        for j in range(4):
                            do = half * 4 + j
                            _mm(nc, aps4[:, j, :],
                                aw2_sb[:, d, do * 128:(do + 1) * 128],
                                ff[:, d, :], start=(d == 0), stop=(d == 7))
                    for j in range(4):
                        do = half * 4 + j
                        if triv:
                            if j < 2:
                                nc.scalar.copy(l2[:, do, :], aps4[:, j, :])
                            else:
                                nc.vector.tensor_copy(l2[:, do, :], aps4[:, j, :])
                        else:
                            nc.vector.tensor_scalar(l2[:, do, :], aps4[:, j, :],
                                                    b2_sb[:, do:do + 1], None, OP.add)

                outa = pw.tile([128, 4, SQ], DT, name="outa")
                outb = pw.tile([128, 4, SQ], DT, name="outb")
                outap = [outa[:, d, :] for d in range(4)] + \
                        [outb[:, d, :] for d in range(4)]
                layernorm_block(st4b, l2, g2_sb, bb2_sb, x1ap, outap, "b")
                nc.sync.dma_start(r_out[:, 0:4, :], outa[:])
                nc.sync.dma_start(r_out[:, 4:8, :], outb[:])

    nc.compile()
    return nc


_NC = {}


def _get_nc(nact, triv):
    key = (nact, triv, _MODE)
    if key not in _NC:
        _NC[key] = build_nc(nact, triv)
    return _NC[key]


def _prep_inputs(inputs):
    """Host-side shard prep: per-core input dicts."""
    x = np.asarray(inputs["batch_x"], np.float32)       # [B, S, D]
    lens = np.asarray(inputs["len_chair"], np.int64)
    wq = np.ascontiguousarray(
        np.asarray(inputs["Wq"], np.float32).transpose(1, 0, 2).reshape(D, D))
    wk = np.ascontiguousarray(
        np.asarray(inputs["Wk"], np.float32).transpose(1, 0, 2).reshape(D, D))
    wv = np.ascontiguousarray(
        np.asarray(inputs["Wv"], np.float32).transpose(1, 0, 2).reshape(D, D))
    com = {
        "wq": wq, "wk": wk, "wv": wv,
        "wo": np.ascontiguousarray(np.asarray(inputs["Wo"], np.float32)),
        "aw1": np.ascontiguousarray(np.asarray(inputs["add1_w"], np.float32)),
        "aw2": np.ascontiguousarray(np.asarray(inputs["add2_w"], np.float32)),
        "fw1": np.ascontiguousarray(np.asarray(inputs["ff_w1"], np.float32)),
        "fw2": np.ascontiguousarray(np.asarray(inputs["ff_w2"], np.float32)),
        "b1c": _chunk(inputs["add1_b"]), "g1c": _chunk(inputs["ln1_g"]),
        "bb1c": _chunk(inputs["ln1_b"]), "fb1c": _chunk(inputs["ff_b1"]),
        "fb2c": _chunk(inputs["ff_b2"]), "b2c": _chunk(inputs["add2_b"]),
        "g2c": _chunk(inputs["ln2_g"]), "bb2c": _chunk(inputs["ln2_b"]),
    }
    xT = [np.ascontiguousarray(x[b].T) for b in range(B)]   # [D, S]
    masks = []
    for b in range(B):
        m = np.where(np.arange(S) >= lens[b], np.float32(MASK_NEG),
                     np.float32(0.0)).astype(np.float32)
        masks.append(np.ascontiguousarray(m.reshape(S // 128, 128).T))
    in_maps = []
    for c in range(NCORES):
        b, q = c // 4, c % 4
        m = dict(com)
        m["xT"] = xT[b]
        m["xq"] = np.ascontiguousarray(xT[b][:, q * SQ:(q + 1) * SQ])
        m["maskb"] = masks[b]
        in_maps.append(m)
    return in_maps


def _chunk(v):
    v = np.asarray(v, np.float32)
    return np.ascontiguousarray(v.reshape(-1, 128).T)


DT_KEYS = ("xT", "xq", "wq", "wk", "wv", "wo", "aw1", "aw2", "fw1", "fw2")


def kernel(trace=False, **inputs):
    _install_ntff_hook()
    lens = np.asarray(inputs["len_chair"], np.int64)
    nact = int(max(1, min(S // 128, -(-int(lens.max()) // 128))))
    triv = (np.all(np.asarray(inputs["ln1_g"]) == 1.0)
            and np.all(np.asarray(inputs["ln2_g"]) == 1.0)
            and not np.any(np.asarray(inputs["ln1_b"]))
            and not np.any(np.asarray(inputs["ln2_b"]))
            and not np.any(np.asarray(inputs["add1_b"]))
            and not np.any(np.asarray(inputs["add2_b"]))
            and not np.any(np.asarray(inputs["ff_b1"]))
            and not np.any(np.asarray(inputs["ff_b2"])))
    nc = _get_nc(nact, triv)
    in_maps = _prep_inputs(inputs)
    import ml_dtypes
    np_dt = mybir.dt.np(DT)
    cache = {}

    def _cast(a, dtype):
        key = (id(a), np.dtype(dtype).str)
        if key not in cache:
            cache[key] = np.ascontiguousarray(a.astype(dtype))
        return cache[key]

    for m in in_maps:
        if np_dt != np.float32:
            for k in DT_KEYS:
                m[k] = _cast(m[k], np_dt)
    res = run_bass_kernel_spmd(nc, in_maps, core_ids=list(range(NCORES)),
                               trace=trace)
    out = np.empty((B, S, D), np.float32)
    for c in range(NCORES):
        b, q = c // 4, c % 4
        out[b, q * SQ:(q + 1) * SQ, :] = \
            np.asarray(res.results[c]["out"]).astype(np.float32).T
    kernel.last_exec_time_ns = res.exec_time_ns
    return out


# revision 52
# speedup vs baseline: 1.0584x; 1.0218x over previous
"""Trainium2 Bass kernel for nn_Encoder_Block (B=2,S=2048,D=1024,H=16,FF=4096).

Sharding: 8 cores, core c -> (batch b=c//4, query block q=c%4 of 512 tokens).
Each core recomputes K/V for its whole batch (no cross-core collectives),
everything else is perfectly sharded. Host does transposes and gather.

Device layout: activations kept transposed [feature, token] throughout, so
every matmul in the chain is a natural lhsT/rhs pair with K=128 contraction
chunks and N=512 moving dim. Attention computes transposed scores [t, sq];
softmax normalizer rides along the PV matmul as a ones-column in V (M=65).
Masking + 1/sqrt(dh) scaling are folded into the Exp activation (bias/scale).
No max-subtraction: scores are O(1) by construction, exp is safe in fp32.

v2: bf16 matmul path, fully-masked key chunks skipped (program specialized
on ceil(max(len)/128) at runtime), dedicated early weight prefetch, fast
approx reciprocal, trivial-LN fast path (g==1, b==0 checked host-side),
broadcasts done as K=1 matmuls on the (otherwise idle) PE.
"""
import sys, types, os
sys.path.insert(0, "/opt/trn_rl_repo")
import numpy as np
from contextlib import ExitStack

import concourse.bass as bass
import concourse.tile as tile
from concourse import bacc, mybir
from concourse.bass_utils import run_bass_kernel_spmd

B, S, D, H, FF = 2, 2048, 1024, 16, 4096
DH = D // H            # 64
SQ = 512               # query tokens per core
NCORES = 8
EPS = 1e-5
MASK_NEG = -60.0       # exp(-60) ~ 8.8e-27 => masked keys contribute ~0

F32 = mybir.dt.float32
F32R = mybir.dt.float32r
# PE compute dtype for matmul-fed tensors:
#   bf16 : 1 cyc/row, half DMA/LDWEIGHTS cost, ~5e-3 output error
#   f32r : 1 cyc/row TF32-like, ~3.5e-4 output error
_MODE = os.environ.get("KERNEL_MM_DT", "bf16")
DT = {"f32r": mybir.dt.float32r, "f32": mybir.dt.float32,
      "bf16": mybir.dt.bfloat16}[_MODE]


def _f(ap):
    # f32r tiles must be bitcast to f32 for DVE/ACT reads; bf16 is native
    if DT == mybir.dt.float32r:
        return ap.bitcast(F32)
    return ap


def _install_ntff_hook():
    """The image's antenv lacks axon_hooks; shim it so trace=True works."""
    try:
        import antenv.axon_hooks  # noqa
        return
    except ImportError:
        pass
    try:
        from trn_agent_boot.trn_boot import _ntff_profile_via_ctypes
        import antenv
        mod = types.ModuleType("antenv.axon_hooks")
        hook = _ntff_profile_via_ctypes("/opt/axon/libaxon_pjrt.so")
        mod.get_axon_ntff_profile_hook = lambda: hook
        mod.set_axon_ntff_profile_hook = lambda h: None
        sys.modules["antenv.axon_hooks"] = mod
        antenv.axon_hooks = mod
    except Exception:
        pass


def _mm(nc, out, lhsT, rhs, start, stop, tile_position=None):
    nc.tensor.matmul(out, lhsT, rhs,
                     start=start, stop=stop, tile_position=tile_position)


def build_nc(nact=16, triv=True):
    """nact: number of active 128-key chunks (fully-masked tail skipped).
    triv: all LN gammas are 1, all betas/biases 0 (checked host-side)."""
    nc = bacc.Bacc(trn_type="TRN2", target_bir_lowering=False, debug=False,
                   num_devices=NCORES, dynamic_dma_scratch_size=512)
    AF = mybir.ActivationFunctionType
    OP = mybir.AluOpType

    # sc groups of up to 4 active 128-token chunks each
    SCG = []
    rem = nact
    while rem > 0:
        SCG.append(min(4, rem))
        rem -= 4

    # ---- DRAM I/O (per-core; program identical across cores) ----
    d_xT = nc.dram_tensor("xT", [D, S], DT, kind="ExternalInput")
    d_xq = nc.dram_tensor("xq", [D, SQ], DT, kind="ExternalInput")
    d_mask = nc.dram_tensor("maskb", [128, S // 128], F32, kind="ExternalInput")
    d_wq = nc.dram_tensor("wq", [D, D], DT, kind="ExternalInput")
    d_wk = nc.dram_tensor("wk", [D, D], DT, kind="ExternalInput")
    d_wv = nc.dram_tensor("wv", [D, D], DT, kind="ExternalInput")
    d_wo = nc.dram_tensor("wo", [D, D], DT, kind="ExternalInput")
    d_aw1 = nc.dram_tensor("aw1", [D, D], DT, kind="ExternalInput")
    d_aw2 = nc.dram_tensor("aw2", [D, D], DT, kind="ExternalInput")
    d_fw1 = nc.dram_tensor("fw1", [D, FF], DT, kind="ExternalInput")
    d_fw2 = nc.dram_tensor("fw2", [FF, D], DT, kind="ExternalInput")
    d_b1 = nc.dram_tensor("b1c", [128, 8], F32, kind="ExternalInput")
    d_g1 = nc.dram_tensor("g1c", [128, 8], F32, kind="ExternalInput")
    d_bb1 = nc.dram_tensor("bb1c", [128, 8], F32, kind="ExternalInput")
    d_fb1 = nc.dram_tensor("fb1c", [128, 32], F32, kind="ExternalInput")
    d_fb2 = nc.dram_tensor("fb2c", [128, 8], F32, kind="ExternalInput")
    d_b2 = nc.dram_tensor("b2c", [128, 8], F32, kind="ExternalInput")
    d_g2 = nc.dram_tensor("g2c", [128, 8], F32, kind="ExternalInput")
    d_bb2 = nc.dram_tensor("bb2c", [128, 8], F32, kind="ExternalInput")
    d_out = nc.dram_tensor("out", [D, SQ], DT, kind="ExternalOutput")

    r_xT = d_xT.ap().rearrange("(c p) s -> p c s", p=128)     # [128, 8, S]
    r_xq = d_xq.ap().rearrange("(c p) s -> p c s", p=128)     # [128, 8, SQ]
    r_wq = d_wq.ap().rearrange("(c p) n -> p c n", p=128)
    r_wk = d_wk.ap().rearrange("(c p) n -> p c n", p=128)
    r_wv = d_wv.ap().rearrange("(c p) n -> p c n", p=128)
    r_wo = d_wo.ap().rearrange("(c p) n -> p c n", p=128)
    r_aw1 = d_aw1.ap().rearrange("(c p) n -> p c n", p=128)
    r_aw2 = d_aw2.ap().rearrange("(c p) n -> p c n", p=128)
    r_fw1 = d_fw1.ap().rearrange("(c p) n -> p c n", p=128)   # [128, 8, FF]
    r_fw2 = d_fw2.ap().rearrange("(c p) n -> p c n", p=128)   # [128, 32, D]
    r_out = d_out.ap().rearrange("(c p) s -> p c s", p=128)

    with tile.TileContext(nc) as tc:
      with ExitStack() as top:
        # one packed const tile (tiles pad to 4KB/partition each otherwise):
        # cols 0:16 maskbias, 16:80 ones, 80:208 sel_e, 208:336 sel_o,
        # 336:464 f32-ones row (for K=1 broadcast matmuls)
        const = top.enter_context(tc.tile_pool(name="const", bufs=1))
        cst = const.tile([128, 468], F32, name="cst")
        mask_sb = cst[:, 0:16]
        ones_f = cst[:, 16:80]
        ones_r = cst[:, 336:464]
        eps_c = cst[0:1, 464:465]
        nc.vector.memset(eps_c, EPS)
        nc.sync.dma_start(mask_sb, d_mask.ap())
        nc.vector.memset(cst[:, 16:336], 0.0)
        nc.vector.memset(ones_f, 1.0)
        nc.vector.memset(ones_r, 1.0)
        sel_e = cst[:, 80:208]
        sel_o = cst[:, 208:336]
        nc.vector.memset(sel_e[0:1, 0:64], 1.0)
        nc.vector.memset(sel_e[32:33, 64:128], 1.0)
        nc.vector.memset(sel_o[64:65, 0:64], 1.0)
        nc.vector.memset(sel_o[96:97, 64:128], 1.0)
        ones1_t = const.tile([1, 128], F32R, name="ones1")
        nc.vector.tensor_copy(ones1_t[:], ones_r[0:1, :])
        ones1 = ones1_t[:]                       # [1,128] lhsT for broadcasts
        ones_sb = const.tile([128, 1], DT, name="ones")
        nc.vector.tensor_copy(ones_sb[:], ones_f[:, 0:1])

        p_x1 = top.enter_context(tc.tile_pool(name="px1", bufs=1))
        # first half of fw1 prefetched from kernel start (consumed in stage 4)
        pfw1a = top.enter_context(tc.tile_pool(name="pfw1a", bufs=1))
        fw1a = pfw1a.tile([128, 8, FF // 2], DT, name="fw1a")
        # dedicated early-prefetch pools: post-attention weights + residual
        ppost = top.enter_context(tc.tile_pool(name="ppost", bufs=1))
        wo_sb = ppost.tile([128, 8, D], DT, name="wo")
        aw1_sb = ppost.tile([128, 8, D], DT, name="aw1")
        xq2_sb = ppost.tile([128, 8, SQ], DT, name="xq2")

        def layernorm_block(st, src_sb, gc, bc, res_aps, dst_aps, pref):
            """dst[d] = LN(src)*g + b + res[d]; src [128,8,SQ], dst/res are
            8 per-chunk APs (split tiles let consumers start early)."""
            pln = st.enter_context(tc.tile_pool(name=pref + "ln", bufs=1))
            pps = st.enter_context(tc.tile_pool(name=pref + "lps", bufs=1, space="PSUM"))
            sq_sb = pln.tile([128, 8, SQ], DT, name=pref + "sq")
            for d in range(8):
                nc.vector.tensor_mul(sq_sb[:, d, :], _f(src_sb[:, d, :]),
                                     _f(src_sb[:, d, :]))
            ps_s = pps.tile([1, SQ], F32, name=pref + "ps_s")
            ps_q = pps.tile([1, SQ], F32, name=pref + "ps_q")
            for d in range(8):
                _mm(nc, ps_s[:], ones_sb[:], src_sb[:, d, :],
                    start=(d == 0), stop=(d == 7))
            for d in range(8):
                _mm(nc, ps_q[:], ones_sb[:], sq_sb[:, d, :],
                    start=(d == 0), stop=(d == 7))
            # mu broadcast first: the apply subtracts can start while the
            # variance -> rstd chain is still running
            bc2 = pln.tile([1, 2, SQ], F32R, name=pref + "bc2")
            nc.scalar.mul(bc2[:, 1, :], ps_s[:], 1.0 / D)
            mu = bc2[:, 1, :].bitcast(F32)
            msq = pln.tile([1, SQ], F32, name=pref + "msq")
            nc.scalar.mul(msq[:], ps_q[:], 1.0 / D)
            ps_b = pps.tile([128, 2, SQ], F32, name=pref + "ps_b")
            _mm(nc, ps_b[:, 1, :], ones1, bc2[:, 1, :],
                start=True, stop=True)
            bsb = pln.tile([128, 2, SQ], DT, name=pref + "bsb")
            nc.scalar.copy(bsb[:, 1, :], ps_b[:, 1, :])
            # var = (msq + eps) - mu^2; rstd = sqrt(1/var) via fast DVE
            # approx reciprocal (~4e-6 rel) + the pre-loaded Sqrt table
            mumul = pln.tile([1, SQ], F32, name=pref + "mm")
            nc.vector.tensor_mul(mumul[:], mu, mu)
            var = pln.tile([1, SQ], F32, name=pref + "var")
            nc.vector.scalar_tensor_tensor(var[:], msq[:], EPS, mumul[:],
                                           OP.add, OP.subtract)
            rv = pln.tile([1, SQ], F32, name=pref + "rv")
            nc.vector.reciprocal_approx_fast(out=rv[:], in_=var[:])
            nc.scalar.activation(bc2[:, 0, :], rv[:], AF.Sqrt)
            _mm(nc, ps_b[:, 0, :], ones1, bc2[:, 0, :],
                start=True, stop=True)
            nc.scalar.copy(bsb[:, 0, :], ps_b[:, 0, :])
            tmp = pln.tile([128, 4, SQ], DT, name=pref + "tmp")
            for d in range(8):
                t = tmp[:, d % 4, :]
                nc.vector.tensor_sub(t, _f(src_sb[:, d, :]), _f(bsb[:, 1, :]))
                nc.vector.tensor_mul(t, _f(t), _f(bsb[:, 0, :]))
                if triv:
                    nc.vector.tensor_add(dst_aps[d], _f(t), _f(res_aps[d]))
                else:
                    nc.vector.tensor_scalar(t, _f(t),
                                            gc[:, d:d + 1], bc[:, d:d + 1],
                                            OP.mult, OP.add)
                    nc.vector.tensor_add(dst_aps[d], _f(t), _f(res_aps[d]))

        # ============ Stages 1-3 ============
        with ExitStack() as s13:
            p_acc = s13.enter_context(tc.tile_pool(name="acc", bufs=1))
            acc = p_acc.tile([128, 8, SQ], DT, name="acc")
            # softmax denominators at partition 32*(h%4), free idx h//4;
            # init 1.0 so unused rows stay finite through the reciprocal
            nrm = p_acc.tile([128, 4, SQ], F32, name="nrm")
            nc.vector.memset(nrm[:], 1.0)

            pwkv = s13.enter_context(tc.tile_pool(name="pwkv", bufs=1))
            wk_sb = pwkv.tile([128, 8, D], DT, name="wk")
            wv_sb = pwkv.tile([128, 8, D], DT, name="wv")
            pxsc = s13.enter_context(tc.tile_pool(name="pxsc", bufs=2))

            with ExitStack() as A:
                p_qT = A.enter_context(tc.tile_pool(name="qT", bufs=1))
                qT = p_qT.tile([128, 8, SQ], DT, name="qT")

                # ---- Stage 1a: Q^T projection ----
                with ExitStack() as st:
                    pw = st.enter_context(tc.tile_pool(name="pwq", bufs=1))
                    px = st.enter_context(tc.tile_pool(name="pxq", bufs=1))
                    pp = st.enter_context(tc.tile_pool(name="ppq", bufs=2, space="PSUM"))
                    wq_sb = pw.tile([128, 8, D], DT, name="wq")
                    xq_sb = px.tile([128, 8, SQ], DT, name="xqp")
                    for d in range(8):
                        nc.sync.dma_start(wq_sb[:, d, :], r_wq[:, d, :])
                        nc.sync.dma_start(xq_sb[:, d, :], r_xq[:, d, :])
                    for p in range(8):
                        ps = pp.tile([128, SQ], F32, name="psq")
                        for d in range(8):
                            _mm(nc, ps[:], wq_sb[:, d, p * 128:(p + 1) * 128],
                                xq_sb[:, d, :], start=(d == 0), stop=(d == 7))
                        nc.scalar.copy(qT[:, p, :], ps[:])

                # K/V weights are first needed after Q-proj: issue their DMAs
                # behind the Q-proj inputs so the PE can start ~10us earlier
                for d in range(8):
                    nc.sync.dma_start(wk_sb[:, d, :], r_wk[:, d, :])
                for d in range(8):
                    nc.sync.dma_start(wv_sb[:, d, :], r_wv[:, d, :])

                # pre-issue the first two attention x-chunk DMAs so they beat
                # the big prefetch burst below in scheduler priority order
                xs_tiles = {}

                def issue_xs(sc):
                    t = pxsc.tile([128, 8, 512], DT, name="xsc", tag="xsc")
                    nt = SCG[sc] * 128
                    for d in range(8):
                        nc.sync.dma_start(t[:, d, 0:nt],
                                          r_xT[:, d, sc * 512:sc * 512 + nt])
                    return t

                xs_tiles[0] = issue_xs(0)
                if len(SCG) > 1:
                    xs_tiles[1] = issue_xs(1)

                # prefetch burst: post-attention weights + residual + fw1 half
                # (issued after everything attention-critical)
                for do in range(8):
                    nc.sync.dma_start(wo_sb[:, :, do * 128:(do + 1) * 128],
                                      r_wo[:, :, do * 128:(do + 1) * 128])
                    nc.sync.dma_start(aw1_sb[:, :, do * 128:(do + 1) * 128],
                                      r_aw1[:, :, do * 128:(do + 1) * 128])
                for d in range(8):
                    nc.sync.dma_start(xq2_sb[:, d, :], r_xq[:, d, :])
                for fh in range(4):
                    nc.sync.dma_start(
                        fw1a[:, :, fh * 512:(fh + 1) * 512],
                        r_fw1[:, :, fh * 512:(fh + 1) * 512])

                # ---- Stage 1b+2: K/V proj + attention, flash over scg ----
                pkv = A.enter_context(tc.tile_pool(name="pkv", bufs=2))
                pexp = A.enter_context(tc.tile_pool(name="pexp", bufs=4))
                aps = A.enter_context(ExitStack())
                psc = aps.enter_context(tc.tile_pool(name="psc", bufs=2, space="PSUM"))
                # dedicated K/V-projection psum: keeps next-chunk projection
                # matmuls runnable while scores/PV own the psc/ppv banks
                pkvp = aps.enter_context(tc.tile_pool(name="pkvp", bufs=2, space="PSUM"))
                ppv = aps.enter_context(tc.tile_pool(name="ppv", bufs=1, space="PSUM"))

                aps2 = A.enter_context(ExitStack())
                for sc, ntc in enumerate(SCG):
                    nt = ntc * 128
                    xs = xs_tiles.pop(sc)
                    if sc + 2 < len(SCG):
                        xs_tiles[sc + 2] = issue_xs(sc + 2)

                    kT = pkv.tile([128, 8, 512], DT, name="kT")
                    for p in range(8):
                        ps = pkvp.tile([128, SQ], F32, name="kvps")
                        for d in range(8):
                            _mm(nc, ps[:, 0:nt], wk_sb[:, d, p * 128:(p + 1) * 128],
                                xs[:, d, 0:nt], start=(d == 0), stop=(d == 7))
                        nc.vector.tensor_copy(kT[:, p, 0:nt], ps[:, 0:nt])

                    vt = pkv.tile([128, 4, 16, 65], DT, name="vt")
                    nc.vector.tensor_copy(
                        vt[:, 0:ntc, :, 64:65],
                        ones_f[:, 0:ntc * 16].rearrange(
                            "p (a b c) -> p a b c", a=ntc, b=16))
                    for i in range(ntc):
                        for nb in range(2):
                            ps = pkvp.tile([128, SQ], F32, name="kvps")
                            for d in range(8):
                                _mm(nc, ps[:], xs[:, d, i * 128:(i + 1) * 128],
                                    wv_sb[:, d, nb * 512:(nb + 1) * 512],
                                    start=(d == 0), stop=(d == 7))
                            nc.vector.tensor_copy(
                                vt[:, i, nb * 8:(nb + 1) * 8, 0:64],
                                ps[:].rearrange("p (h e) -> p h e", e=64))

                    if sc == len(SCG) - 1:
                        # last chunk group: no further K/V projection exists
                        # to fill drain stalls, so retire the projection psum
                        # and run with double-buffered PV accumulators
                        aps.close()
                        psc_u = aps2.enter_context(
                            tc.tile_pool(name="psc2", bufs=2, space="PSUM"))
                        ppv_u = aps2.enter_context(
                            tc.tile_pool(name="ppv2", bufs=2, space="PSUM"))
                    else:
                        psc_u, ppv_u = psc, ppv

                    for p in range(8):
                        h0, h1 = 2 * p, 2 * p + 1
                        pva = ppv_u.tile([128, 2, SQ], F32, name="pva")
                        for i in range(ntc):
                            tci = sc * 4 + i
                            s01 = psc_u.tile([128, 2, SQ], F32, name="s01")
                            _mm(nc, s01[:, 0, :],
                                kT[0:64, p, i * 128:(i + 1) * 128],
                                qT[0:64, p, :], start=True, stop=True,
                                tile_position=(0, 0))
                            _mm(nc, s01[:, 1, :],
                                kT[64:128, p, i * 128:(i + 1) * 128],
                                qT[64:128, p, :], start=True, stop=True,
                                tile_position=(64, 0))
                            e01 = pexp.tile([128, 2, SQ], DT, name="e01")
                            nc.scalar.activation(e01[:], s01[:], AF.Exp,
                                                 bias=mask_sb[:, tci:tci + 1],
                                                 scale=0.125)
                            _mm(nc, pva[0:65, 0, :], vt[:, i, h0, :], e01[:, 0, :],
                                start=(i == 0), stop=(i == ntc - 1))
                            _mm(nc, pva[0:65, 1, :], vt[:, i, h1, :], e01[:, 1, :],
                                start=(i == 0), stop=(i == ntc - 1))
                        a0, c0 = 32 * (h0 % 4), h0 // 4
                        a1, c1 = 32 * (h1 % 4), h1 // 4
                        if sc == 0:
                            nc.vector.tensor_copy(acc[0:64, p, :], pva[0:64, 0, :])
                            nc.vector.tensor_copy(acc[64:128, p, :], pva[0:64, 1, :])
                            nc.vector.tensor_copy(nrm[a0:a0 + 1, c0, :], pva[64:65, 0, :])
                            nc.vector.tensor_copy(nrm[a1:a1 + 1, c1, :], pva[64:65, 1, :])
                        else:
                            nc.vector.tensor_add(acc[0:64, p, :],
                                                 _f(acc[0:64, p, :]), pva[0:64, 0, :])
                            nc.vector.tensor_add(acc[64:128, p, :],
                                                 _f(acc[64:128, p, :]), pva[0:64, 1, :])
                            nc.vector.tensor_add(nrm[a0:a0 + 1, c0, :],
                                                 nrm[a0:a0 + 1, c0, :], pva[64:65, 0, :])
                            nc.vector.tensor_add(nrm[a1:a1 + 1, c1, :],
                                                 nrm[a1:a1 + 1, c1, :], pva[64:65, 1, :])

                # normalize: acc[:, p, :] *= 1/nrm via selector-matmul bcast;
                # reciprocal per c-group so it pipelines with the tail PVs
                aps2.close()
                ppb = A.enter_context(tc.tile_pool(name="ppb", bufs=2, space="PSUM"))
                pnr = A.enter_context(tc.tile_pool(name="pnr", bufs=2))
                for c in range(4):
                    nc.vector.reciprocal_approx_fast(
                        out=nrm[:, c, :], in_=nrm[:, c, :])
                # pre-load the Sqrt ACT table set now (all attention exps are
                # done); keyed off the last reciprocal so the scheduler
                # cannot hoist the table swap above the exps
                scr = pnr.tile([1, 1], F32, name="scr")
                nc.scalar.activation(scr[:], nrm[0:1, 3, 0:1], AF.Sqrt)
                for p in range(8):
                    sel = sel_e if p % 2 == 0 else sel_o
                    ps_rb = ppb.tile([128, SQ], F32, name="ps_rb")
                    nc.tensor.matmul(ps_rb[:], sel, nrm[:, p // 2, :],
                                     start=True, stop=True)
                    nc.vector.tensor_mul(acc[:, p, :], _f(acc[:, p, :]), ps_rb[:])

            # ---- Stage 3: Wo + add1 + LN1 + residual ----
            with ExitStack() as st:
                pw = st.enter_context(tc.tile_pool(name="pw3", bufs=1))
                b1_sb = pw.tile([128, 8], F32, name="b1")
                g1_sb = pw.tile([128, 8], F32, name="g1")
                bb1_sb = pw.tile([128, 8], F32, name="bb1")
                if not triv:
                    nc.sync.dma_start(b1_sb[:], d_b1.ap())
                    nc.sync.dma_start(g1_sb[:], d_g1.ap())
                    nc.sync.dma_start(bb1_sb[:], d_bb1.ap())

                # x1 split into two tiles so FFN1 can start on the first half
                # while the LN apply chain still writes the second
                x1a = p_x1.tile([128, 4, SQ], DT, name="x1a")
                x1b = p_x1.tile([128, 4, SQ], DT, name="x1b")
                x1ap = [x1a[:, d, :] for d in range(4)] + \
                       [x1b[:, d, :] for d in range(4)]
                pao = st.enter_context(tc.tile_pool(name="pao", bufs=1))
                ao = pao.tile([128, 8, SQ], DT, name="ao")
                padd = st.enter_context(tc.tile_pool(name="padd1", bufs=1, space="PSUM"))
                with ExitStack() as stW:
                    pp = stW.enter_context(tc.tile_pool(name="pp3", bufs=2, space="PSUM"))
                    for do in range(8):
                        ps = pp.tile([128, SQ], F32, name="ps3a")
                        for d in range(8):
                            _mm(nc, ps[:], wo_sb[:, d, do * 128:(do + 1) * 128],
                                acc[:, d, :], start=(d == 0), stop=(d == 7))
                        nc.scalar.copy(ao[:, do, :], ps[:])
                # add1 in two 4-bank d-outer passes: pass MMs interleave with
                # the Wo do-loop above as its ao chunks land
                l1 = pao.tile([128, 8, SQ], DT, name="l1")
                for half in range(2):
                    aps4 = padd.tile([128, 4, SQ], F32, name="a1ps", tag="a1ps")
                    for d in range(8):
                        for j in range(4):
                            do = half * 4 + j
                            _mm(nc, aps4[:, j, :],
                                aw1_sb[:, d, do * 128:(do + 1) * 128],
                                ao[:, d, :], start=(d == 0), stop=(d == 7))
                    for j in range(4):
                        do = half * 4 + j
                        if triv:
                            eng = nc.scalar if j < 2 else nc.vector
                            if j < 2:
                                eng.copy(l1[:, do, :], aps4[:, j, :])
                            else:
                                eng.tensor_copy(l1[:, do, :], aps4[:, j, :])
                        else:
                            nc.vector.tensor_scalar(l1[:, do, :], aps4[:, j, :],
                                                    b1_sb[:, do:do + 1], None, OP.add)
                layernorm_block(st, l1, g1_sb, bb1_sb,
                                [xq2_sb[:, d, :] for d in range(8)], x1ap, "a")

        # ================= Stage 4: FFN + add2 + LN2 + residual =================
        with ExitStack() as st:
            pff = st.enter_context(tc.tile_pool(name="pff", bufs=1))
            ff = pff.tile([128, 8, SQ], DT, name="ff")
            aw2_sb = pff.tile([128, 8, D], DT, name="aw2")
            for do in range(8):
                nc.sync.dma_start(aw2_sb[:, :, do * 128:(do + 1) * 128],
                                  r_aw2[:, :, do * 128:(do + 1) * 128])
            padd2 = st.enter_context(tc.tile_pool(name="padd2", bufs=1, space="PSUM"))
            with ExitStack() as st4a:
                ph = st4a.enter_context(tc.tile_pool(name="ph", bufs=1))
                h_sb = ph.tile([128, 32, SQ], DT, name="h")
                pwc = st4a.enter_context(tc.tile_pool(name="pwc", bufs=6))
                pwc2 = st4a.enter_context(tc.tile_pool(name="pwc2", bufs=4))
                fb1_sb = ph.tile([128, 32], F32, name="fb1")
                fb2_sb = ph.tile([128, 8], F32, name="fb2")
                if not triv:
                    nc.sync.dma_start(fb1_sb[:], d_fb1.ap())
                    nc.sync.dma_start(fb2_sb[:], d_fb2.ap())
                pp = st4a.enter_context(tc.tile_pool(name="pp4", bufs=2, space="PSUM"))

                for f in range(32):
                    if f < 16:
                        w1t = fw1a[:, :, f * 128:(f + 1) * 128]
                    else:
                        w1c = pwc.tile([128, 8, 128], DT, name="w1c")
                        nc.sync.dma_start(w1c[:], r_fw1[:, :, f * 128:(f + 1) * 128])
                        w1t = w1c[:]
                    ps = pp.tile([128, SQ], F32, name="ps4a")
                    for d in range(8):
                        _mm(nc, ps[:], w1t[:, d, :], x1ap[d],
                            start=(d == 0), stop=(d == 7))
                    if triv:
                        nc.vector.tensor_scalar(h_sb[:, f, :], ps[:],
                                                0.0, None, OP.max)
                    else:
                        nc.vector.tensor_scalar(h_sb[:, f, :], ps[:],
                                                fb1_sb[:, f:f + 1], 0.0,
                                                OP.add, OP.max)

                for do in range(8):
                    w2t = pwc2.tile([128, 32, 128], DT, name="w2c")
                    nc.sync.dma_start(w2t[:], r_fw2[:, :, do * 128:(do + 1) * 128])
                    ps = pp.tile([128, SQ], F32, name="ps4b")
                    for f in range(32):
                        _mm(nc, ps[:], w2t[:, f, :], h_sb[:, f, :],
                            start=(f == 0), stop=(f == 31))
                    if triv:
                        nc.scalar.copy(ff[:, do, :], ps[:])
                    else:
                        nc.vector.tensor_scalar(ff[:, do, :], ps[:],
                                                fb2_sb[:, do:do + 1], None, OP.add)

            with ExitStack() as st4b:
                pw = st4b.enter_context(tc.tile_pool(name="pw4", bufs=1))
                b2_sb = pw.tile([128, 8], F32, name="b2")
                g2_sb = pw.tile([128, 8], F32, name="g2")
                bb2_sb = pw.tile([128, 8], F32, name="bb2")
                if not triv:
                    nc.sync.dma_start(b2_sb[:], d_b2.ap())
                    nc.sync.dma_start(g2_sb[:], d_g2.ap())
                    nc.sync.dma_start(bb2_sb[:], d_bb2.ap())

                # add2 in two 4-bank d-outer passes: pass-A MMs interleave
                # with the FFN2 do-loop as its ff chunks land
                l2 = pw.tile([128, 8, SQ], DT, name="l2")
                for half in range(2):
                    aps4 = padd2.tile([128, 4, SQ], F32, name="a2ps", tag="a2ps")
                    for d in range(8):
                        for j in range(4):
                            do = half * 4 + j
                            _mm(nc, aps4[:, j, :],
                                aw2_sb[:, d, do * 128:(do + 1) * 128],
                                ff[:, d, :], start=(d == 0), stop=(d == 7))
                    for j in range(4):
                        do = half * 4 + j
                        if triv:
                            if j < 2:
                                nc.scalar.copy(l2[:, do, :], aps4[:, j, :])
                            else:
                                nc.vector.tensor_copy(l2[:, do, :], aps4[:, j, :])
                        else:
                            nc.vector.tensor_scalar(l2[:, do, :], aps4[:, j, :],
                                                    b2_sb[:, do:do + 1], None, OP.add)

                outa = pw.tile([128, 4, SQ], DT, name="outa")
                outb = pw.tile([128, 4, SQ], DT, name="outb")
                outap = [outa[:, d, :] for d in range(4)] + \
                        [outb[:, d, :] for d in range(4)]
                layernorm_block(st4b, l2, g2_sb, bb2_sb, x1ap, outap, "b")
                nc.sync.dma_start(r_out[:, 0:4, :], outa[:])
                nc.sync.dma_start(r_out[:, 4:8, :], outb[:])

    nc.compile()
    return nc


_NC = {}


def _get_nc(nact, triv):
    key = (nact, triv, _MODE)
    if key not in _NC:
        _NC[key] = build_nc(nact, triv)
    return _NC[key]


def _prep_inputs(inputs):
    """Host-side shard prep: per-core input dicts."""
    x = np.asarray(inputs["batch_x"], np.float32)       # [B, S, D]
    lens = np.asarray(inputs["len_chair"], np.int64)
    wq = np.ascontiguousarray(
        np.asarray(inputs["Wq"], np.float32).transpose(1, 0, 2).reshape(D, D))
    wk = np.ascontiguousarray(
        np.asarray(inputs["Wk"], np.float32).transpose(1, 0, 2).reshape(D, D))
    wv = np.ascontiguousarray(
        np.asarray(inputs["Wv"], np.float32).transpose(1, 0, 2).reshape(D, D))
    com = {
        "wq": wq, "wk": wk, "wv": wv,
        "wo": np.ascontiguousarray(np.asarray(inputs["Wo"], np.float32)),
        "aw1": np.ascontiguousarray(np.asarray(inputs["add1_w"], np.float32)),
        "aw2": np.ascontiguousarray(np.asarray(inputs["add2_w"], np.float32)),
        "fw1": np.ascontiguousarray(np.asarray(inputs["ff_w1"], np.float32)),
        "fw2": np.ascontiguousarray(np.asarray(inputs["ff_w2"], np.float32)),
        "b1c": _chunk(inputs["add1_b"]), "g1c": _chunk(inputs["ln1_g"]),
        "bb1c": _chunk(inputs["ln1_b"]), "fb1c": _chunk(inputs["ff_b1"]),
        "fb2c": _chunk(inputs["ff_b2"]), "b2c": _chunk(inputs["add2_b"]),
        "g2c": _chunk(inputs["ln2_g"]), "bb2c": _chunk(inputs["ln2_b"]),
    }
    xT = [np.ascontiguousarray(x[b].T) for b in range(B)]   # [D, S]
    masks = []
    for b in range(B):
        m = np.where(np.arange(S) >= lens[b], np.float32(MASK_NEG),
                     np.float32(0.0)).astype(np.float32)
        masks.append(np.ascontiguousarray(m.reshape(S // 128, 128).T))
    in_maps = []
    for c in range(NCORES):
        b, q = c // 4, c % 4
        m = dict(com)
        m["xT"] = xT[b]
        m["xq"] = np.ascontiguousarray(xT[b][:, q * SQ:(q + 1) * SQ])
        m["maskb"] = masks[b]
        in_maps.append(m)
    return in_maps


def _chunk(v):
    v = np.asarray(v, np.float32)
    return np.ascontiguousarray(v.reshape(-1, 128).T)


DT_KEYS = ("xT", "xq", "wq", "wk", "wv", "wo", "aw1", "aw2", "fw1", "fw2")


def kernel(trace=False, **inputs):
    _install_ntff_hook()
    lens = np.asarray(inputs["len_chair"], np.int64)
    nact = int(max(1, min(S // 128, -(-int(lens.max()) // 128))))
    triv = (np.all(np.asarray(inputs["ln1_g"]) == 1.0)
            and np.all(np.asarray(inputs["ln2_g"]) == 1.0)
            and not np.any(np.asarray(inputs["ln1_b"]))
            and not np.any(np.asarray(inputs["ln2_b"]))
            and not np.any(np.asarray(inputs["add1_b"]))
            and not np.any(np.asarray(inputs["add2_b"]))
            and not np.any(np.asarray(inputs["ff_b1"]))
            and not np.any(np.asarray(inputs["ff_b2"])))
    nc = _get_nc(nact, triv)
    in_maps = _prep_inputs(inputs)
    import ml_dtypes
    np_dt = mybir.dt.np(DT)
    cache = {}

    def _cast(a, dtype):
        key = (id(a), np.dtype(dtype).str)
        if key not in cache:
            cache[key] = np.ascontiguousarray(a.astype(dtype))
        return cache[key]

    for m in in_maps:
        if np_dt != np.float32:
            for k in DT_KEYS:
                m[k] = _cast(m[k], np_dt)
    res = run_bass_kernel_spmd(nc, in_maps, core_ids=list(range(NCORES)),
                               trace=trace)
    out = np.empty((B, S, D), np.float32)
    for c in range(NCORES):
        b, q = c // 4, c % 4
        out[b, q * SQ:(q + 1) * SQ, :] = \
            np.asarray(res.results[c]["out"]).astype(np.float32).T
    kernel.last_exec_time_ns = res.exec_time_ns
    return out


# revision 54
# speedup vs baseline: 1.0586x; 1.0002x over previous
"""Trainium2 Bass kernel for nn_Encoder_Block (B=2,S=2048,D=1024,H=16,FF=4096).

Sharding: 8 cores, core c -> (batch b=c//4, query block q=c%4 of 512 tokens).
Each core recomputes K/V for its whole batch (no cross-core collectives),
everything else is perfectly sharded. Host does transposes and gather.

Device layout: activations kept transposed [feature, token] throughout, so
every matmul in the chain is a natural lhsT/rhs pair with K=128 contraction
chunks and N=512 moving dim. Attention computes transposed scores [t, sq];
softmax normalizer rides along the PV matmul as a ones-column in V (M=65).
Masking + 1/sqrt(dh) scaling are folded into the Exp activation (bias/scale).
No max-subtraction: scores are O(1) by construction, exp is safe in fp32.

v2: bf16 matmul path, fully-masked key chunks skipped (program specialized
on ceil(max(len)/128) at runtime), dedicated early weight prefetch, fast
approx reciprocal, trivial-LN fast path (g==1, b==0 checked host-side),
broadcasts done as K=1 matmuls on the (otherwise idle) PE.
"""
import sys, types, os
sys.path.insert(0, "/opt/trn_rl_repo")
import numpy as np
from contextlib import ExitStack

import concourse.bass as bass
import concourse.tile as tile
from concourse import bacc, mybir
from concourse.bass_utils import run_bass_kernel_spmd

B, S, D, H, FF = 2, 2048, 1024, 16, 4096
DH = D // H            # 64
SQ = 512               # query tokens per core
NCORES = 8
EPS = 1e-5
MASK_NEG = -60.0       # exp(-60) ~ 8.8e-27 => masked keys contribute ~0

F32 = mybir.dt.float32
F32R = mybir.dt.float32r
# PE compute dtype for matmul-fed tensors:
#   bf16 : 1 cyc/row, half DMA/LDWEIGHTS cost, ~5e-3 output error
#   f32r : 1 cyc/row TF32-like, ~3.5e-4 output error
_MODE = os.environ.get("KERNEL_MM_DT", "bf16")
DT = {"f32r": mybir.dt.float32r, "f32": mybir.dt.float32,
      "bf16": mybir.dt.bfloat16}[_MODE]


def _f(ap):
    # f32r tiles must be bitcast to f32 for DVE/ACT reads; bf16 is native
    if DT == mybir.dt.float32r:
        return ap.bitcast(F32)
    return ap


def _install_ntff_hook():
    """The image's antenv lacks axon_hooks; shim it so trace=True works."""
    try:
        import antenv.axon_hooks  # noqa
        return
    except ImportError:
        pass
    try:
        from trn_agent_boot.trn_boot import _ntff_profile_via_ctypes
        import antenv
        mod = types.ModuleType("antenv.axon_hooks")
        hook = _ntff_profile_via_ctypes("/opt/axon/libaxon_pjrt.so")
        mod.get_axon_ntff_profile_hook = lambda: hook
        mod.set_axon_ntff_profile_hook = lambda h: None
        sys.modules["antenv.axon_hooks"] = mod
        antenv.axon_hooks = mod
    except Exception:
        pass


def _mm(nc, out, lhsT, rhs, start, stop, tile_position=None):
    nc.tensor.matmul(out, lhsT, rhs,
                     start=start, stop=stop, tile_position=tile_position)


def build_nc(nact=16, triv=True):
    """nact: number of active 128-key chunks (fully-masked tail skipped).
    triv: all LN gammas are 1, all betas/biases 0 (checked host-side)."""
    nc = bacc.Bacc(trn_type="TRN2", target_bir_lowering=False, debug=False,
                   num_devices=NCORES, dynamic_dma_scratch_size=512)
    AF = mybir.ActivationFunctionType
    OP = mybir.AluOpType

    # sc groups of up to 4 active 128-token chunks each
    SCG = []
    rem = nact
    while rem > 0:
        SCG.append(min(4, rem))
        rem -= 4

    # ---- DRAM I/O (per-core; program identical across cores) ----
    d_xT = nc.dram_tensor("xT", [D, S], DT, kind="ExternalInput")
    d_xq = nc.dram_tensor("xq", [D, SQ], DT, kind="ExternalInput")
    d_mask = nc.dram_tensor("maskb", [128, S // 128], F32, kind="ExternalInput")
    d_wq = nc.dram_tensor("wq", [D, D], DT, kind="ExternalInput")
    d_wk = nc.dram_tensor("wk", [D, D], DT, kind="ExternalInput")
    d_wv = nc.dram_tensor("wv", [D, D], DT, kind="ExternalInput")
    d_wo = nc.dram_tensor("wo", [D, D], DT, kind="ExternalInput")
    d_aw1 = nc.dram_tensor("aw1", [D, D], DT, kind="ExternalInput")
    d_aw2 = nc.dram_tensor("aw2", [D, D], DT, kind="ExternalInput")
    d_fw1 = nc.dram_tensor("fw1", [D, FF], DT, kind="ExternalInput")
    d_fw2 = nc.dram_tensor("fw2", [FF, D], DT, kind="ExternalInput")
    d_b1 = nc.dram_tensor("b1c", [128, 8], F32, kind="ExternalInput")
    d_g1 = nc.dram_tensor("g1c", [128, 8], F32, kind="ExternalInput")
    d_bb1 = nc.dram_tensor("bb1c", [128, 8], F32, kind="ExternalInput")
    d_fb1 = nc.dram_tensor("fb1c", [128, 32], F32, kind="ExternalInput")
    d_fb2 = nc.dram_tensor("fb2c", [128, 8], F32, kind="ExternalInput")
    d_b2 = nc.dram_tensor("b2c", [128, 8], F32, kind="ExternalInput")
    d_g2 = nc.dram_tensor("g2c", [128, 8], F32, kind="ExternalInput")
    d_bb2 = nc.dram_tensor("bb2c", [128, 8], F32, kind="ExternalInput")
    d_out = nc.dram_tensor("out", [D, SQ], DT, kind="ExternalOutput")

    r_xT = d_xT.ap().rearrange("(c p) s -> p c s", p=128)     # [128, 8, S]
    r_xq = d_xq.ap().rearrange("(c p) s -> p c s", p=128)     # [128, 8, SQ]
    r_wq = d_wq.ap().rearrange("(c p) n -> p c n", p=128)
    r_wk = d_wk.ap().rearrange("(c p) n -> p c n", p=128)
    r_wv = d_wv.ap().rearrange("(c p) n -> p c n", p=128)
    r_wo = d_wo.ap().rearrange("(c p) n -> p c n", p=128)
    r_aw1 = d_aw1.ap().rearrange("(c p) n -> p c n", p=128)
    r_aw2 = d_aw2.ap().rearrange("(c p) n -> p c n", p=128)
    r_fw1 = d_fw1.ap().rearrange("(c p) n -> p c n", p=128)   # [128, 8, FF]
    r_fw2 = d_fw2.ap().rearrange("(c p) n -> p c n", p=128)   # [128, 32, D]
    r_out = d_out.ap().rearrange("(c p) s -> p c s", p=128)

    with tile.TileContext(nc) as tc:
      with ExitStack() as top:
        # one packed const tile (tiles pad to 4KB/partition each otherwise):
        # cols 0:16 maskbias, 16:80 ones, 80:208 sel_e, 208:336 sel_o,
        # 336:464 f32-ones row (for K=1 broadcast matmuls)
        const = top.enter_context(tc.tile_pool(name="const", bufs=1))
        cst = const.tile([128, 468], F32, name="cst")
        mask_sb = cst[:, 0:16]
        ones_f = cst[:, 16:80]
        ones_r = cst[:, 336:464]
        eps_c = cst[0:1, 464:465]
        nc.vector.memset(eps_c, EPS)
        nc.sync.dma_start(mask_sb, d_mask.ap())
        nc.vector.memset(cst[:, 16:336], 0.0)
        nc.vector.memset(ones_f, 1.0)
        nc.vector.memset(ones_r, 1.0)
        sel_e = cst[:, 80:208]
        sel_o = cst[:, 208:336]
        nc.vector.memset(sel_e[0:1, 0:64], 1.0)
        nc.vector.memset(sel_e[32:33, 64:128], 1.0)
        nc.vector.memset(sel_o[64:65, 0:64], 1.0)
        nc.vector.memset(sel_o[96:97, 64:128], 1.0)
        ones1_t = const.tile([1, 128], F32R, name="ones1")
        nc.vector.tensor_copy(ones1_t[:], ones_r[0:1, :])
        ones1 = ones1_t[:]                       # [1,128] lhsT for broadcasts
        ones_sb = const.tile([128, 1], DT, name="ones")
        nc.vector.tensor_copy(ones_sb[:], ones_f[:, 0:1])

        p_x1 = top.enter_context(tc.tile_pool(name="px1", bufs=1))
        # first half of fw1 prefetched from kernel start (consumed in stage 4)
        pfw1a = top.enter_context(tc.tile_pool(name="pfw1a", bufs=1))
        fw1a = pfw1a.tile([128, 8, FF // 2], DT, name="fw1a")
        # dedicated early-prefetch pools: post-attention weights + residual
        ppost = top.enter_context(tc.tile_pool(name="ppost", bufs=1))
        wo_sb = ppost.tile([128, 8, D], DT, name="wo")
        aw1_sb = ppost.tile([128, 8, D], DT, name="aw1")
        xq2_sb = ppost.tile([128, 8, SQ], DT, name="xq2")

        def layernorm_block(st, src_sb, gc, bc, res_aps, dst_aps, pref):
            """dst[d] = LN(src)*g + b + res[d]; src [128,8,SQ], dst/res are
            8 per-chunk APs (split tiles let consumers start early)."""
            pln = st.enter_context(tc.tile_pool(name=pref + "ln", bufs=1))
            pps = st.enter_context(tc.tile_pool(name=pref + "lps", bufs=1, space="PSUM"))
            sq_sb = pln.tile([128, 8, SQ], DT, name=pref + "sq")
            for d in range(8):
                nc.vector.tensor_mul(sq_sb[:, d, :], _f(src_sb[:, d, :]),
                                     _f(src_sb[:, d, :]))
            ps_s = pps.tile([1, SQ], F32, name=pref + "ps_s")
            ps_q = pps.tile([1, SQ], F32, name=pref + "ps_q")
            for d in range(8):
                _mm(nc, ps_s[:], ones_sb[:], src_sb[:, d, :],
                    start=(d == 0), stop=(d == 7))
            for d in range(8):
                _mm(nc, ps_q[:], ones_sb[:], sq_sb[:, d, :],
                    start=(d == 0), stop=(d == 7))
            # mu broadcast first: the apply subtracts can start while the
            # variance -> rstd chain is still running
            bc2 = pln.tile([1, 2, SQ], F32R, name=pref + "bc2")
            nc.scalar.mul(bc2[:, 1, :], ps_s[:], 1.0 / D)
            mu = bc2[:, 1, :].bitcast(F32)
            msq = pln.tile([1, SQ], F32, name=pref + "msq")
            nc.scalar.mul(msq[:], ps_q[:], 1.0 / D)
            ps_b = pps.tile([128, 2, SQ], F32, name=pref + "ps_b")
            _mm(nc, ps_b[:, 1, :], ones1, bc2[:, 1, :],
                start=True, stop=True)
            bsb = pln.tile([128, 2, SQ], DT, name=pref + "bsb")
            nc.scalar.copy(bsb[:, 1, :], ps_b[:, 1, :])
            # var = (msq + eps) - mu^2; rstd = sqrt(1/var) via fast DVE
            # approx reciprocal (~4e-6 rel) + the pre-loaded Sqrt table
            mumul = pln.tile([1, SQ], F32, name=pref + "mm")
            nc.vector.tensor_mul(mumul[:], mu, mu)
            var = pln.tile([1, SQ], F32, name=pref + "var")
            nc.vector.scalar_tensor_tensor(var[:], msq[:], EPS, mumul[:],
                                           OP.add, OP.subtract)
            rv = pln.tile([1, SQ], F32, name=pref + "rv")
            nc.vector.reciprocal_approx_fast(out=rv[:], in_=var[:])
            nc.scalar.activation(bc2[:, 0, :], rv[:], AF.Sqrt)
            _mm(nc, ps_b[:, 0, :], ones1, bc2[:, 0, :],
                start=True, stop=True)
            nc.scalar.copy(bsb[:, 0, :], ps_b[:, 0, :])
            tmp = pln.tile([128, 4, SQ], DT, name=pref + "tmp")
            for d in range(8):
                t = tmp[:, d % 4, :]
                nc.vector.tensor_sub(t, _f(src_sb[:, d, :]), _f(bsb[:, 1, :]))
                nc.vector.tensor_mul(t, _f(t), _f(bsb[:, 0, :]))
                if triv:
                    nc.vector.tensor_add(dst_aps[d], _f(t), _f(res_aps[d]))
                else:
                    nc.vector.tensor_scalar(t, _f(t),
                                            gc[:, d:d + 1], bc[:, d:d + 1],
                                            OP.mult, OP.add)
                    nc.vector.tensor_add(dst_aps[d], _f(t), _f(res_aps[d]))

        # ============ Stages 1-3 ============
        with ExitStack() as s13:
            p_acc = s13.enter_context(tc.tile_pool(name="acc", bufs=1))
            acc = p_acc.tile([128, 8, SQ], DT, name="acc")
            # softmax denominators at partition 32*(h%4), free idx h//4;
            # init 1.0 so unused rows stay finite through the reciprocal
            nrm = p_acc.tile([128, 4, SQ], F32, name="nrm")
            nc.vector.memset(nrm[:], 1.0)

            pwkv = s13.enter_context(tc.tile_pool(name="pwkv", bufs=1))
            wk_sb = pwkv.tile([128, 8, D], DT, name="wk")
            wv_sb = pwkv.tile([128, 8, D], DT, name="wv")
            pxsc = s13.enter_context(tc.tile_pool(name="pxsc", bufs=2))

            with ExitStack() as A:
                p_qT = A.enter_context(tc.tile_pool(name="qT", bufs=1))
                qT = p_qT.tile([128, 8, SQ], DT, name="qT")

                # ---- Stage 1a: Q^T projection ----
                # xq2 doubles as both Q-proj input and the stage-3 residual —
                # loaded once, on the critical path
                with ExitStack() as st:
                    pw = st.enter_context(tc.tile_pool(name="pwq", bufs=1))
                    pp = st.enter_context(tc.tile_pool(name="ppq", bufs=2, space="PSUM"))
                    wq_sb = pw.tile([128, 8, D], DT, name="wq")
                    for d in range(8):
                        nc.sync.dma_start(xq2_sb[:, d, :], r_xq[:, d, :])
                        nc.sync.dma_start(wq_sb[:, d, :], r_wq[:, d, :])
                    for p in range(8):
                        ps = pp.tile([128, SQ], F32, name="psq")
                        for d in range(8):
                            _mm(nc, ps[:], wq_sb[:, d, p * 128:(p + 1) * 128],
                                xq2_sb[:, d, :], start=(d == 0), stop=(d == 7))
                        nc.scalar.copy(qT[:, p, :], ps[:])

                # K/V weights are first needed after Q-proj: issue their DMAs
                # behind the Q-proj inputs so the PE can start ~10us earlier
                for d in range(8):
                    nc.sync.dma_start(wk_sb[:, d, :], r_wk[:, d, :])
                for d in range(8):
                    nc.sync.dma_start(wv_sb[:, d, :], r_wv[:, d, :])

                # pre-issue the first two attention x-chunk DMAs so they beat
                # the big prefetch burst below in scheduler priority order
                xs_tiles = {}

                def issue_xs(sc):
                    t = pxsc.tile([128, 8, 512], DT, name="xsc", tag="xsc")
                    nt = SCG[sc] * 128
                    for d in range(8):
                        nc.sync.dma_start(t[:, d, 0:nt],
                                          r_xT[:, d, sc * 512:sc * 512 + nt])
                    return t

                xs_tiles[0] = issue_xs(0)
                if len(SCG) > 1:
                    xs_tiles[1] = issue_xs(1)

                # prefetch burst: post-attention weights + residual + fw1 half
                # (issued after everything attention-critical)
                for do in range(8):
                    nc.sync.dma_start(wo_sb[:, :, do * 128:(do + 1) * 128],
                                      r_wo[:, :, do * 128:(do + 1) * 128])
                    nc.sync.dma_start(aw1_sb[:, :, do * 128:(do + 1) * 128],
                                      r_aw1[:, :, do * 128:(do + 1) * 128])
                for fh in range(4):
                    nc.sync.dma_start(
                        fw1a[:, :, fh * 512:(fh + 1) * 512],
                        r_fw1[:, :, fh * 512:(fh + 1) * 512])

                # ---- Stage 1b+2: K/V proj + attention, flash over scg ----
                pkv = A.enter_context(tc.tile_pool(name="pkv", bufs=2))
                pexp = A.enter_context(tc.tile_pool(name="pexp", bufs=4))
                aps = A.enter_context(ExitStack())
                psc = aps.enter_context(tc.tile_pool(name="psc", bufs=2, space="PSUM"))
                # dedicated K/V-projection psum: keeps next-chunk projection
                # matmuls runnable while scores/PV own the psc/ppv banks
                pkvp = aps.enter_context(tc.tile_pool(name="pkvp", bufs=2, space="PSUM"))
                ppv = aps.enter_context(tc.tile_pool(name="ppv", bufs=1, space="PSUM"))

                aps2 = A.enter_context(ExitStack())
                for sc, ntc in enumerate(SCG):
                    nt = ntc * 128
                    xs = xs_tiles.pop(sc)
                    if sc + 2 < len(SCG):
                        xs_tiles[sc + 2] = issue_xs(sc + 2)

                    kT = pkv.tile([128, 8, 512], DT, name="kT")
                    for p in range(8):
                        ps = pkvp.tile([128, SQ], F32, name="kvps")
                        for d in range(8):
                            _mm(nc, ps[:, 0:nt], wk_sb[:, d, p * 128:(p + 1) * 128],
                                xs[:, d, 0:nt], start=(d == 0), stop=(d == 7))
                        nc.vector.tensor_copy(kT[:, p, 0:nt], ps[:, 0:nt])

                    vt = pkv.tile([128, 4, 16, 65], DT, name="vt")
                    nc.vector.tensor_copy(
                        vt[:, 0:ntc, :, 64:65],
                        ones_f[:, 0:ntc * 16].rearrange(
                            "p (a b c) -> p a b c", a=ntc, b=16))
                    for i in range(ntc):
                        for nb in range(2):
                            ps = pkvp.tile([128, SQ], F32, name="kvps")
                            for d in range(8):
                                _mm(nc, ps[:], xs[:, d, i * 128:(i + 1) * 128],
                                    wv_sb[:, d, nb * 512:(nb + 1) * 512],
                                    start=(d == 0), stop=(d == 7))
                            nc.vector.tensor_copy(
                                vt[:, i, nb * 8:(nb + 1) * 8, 0:64],
                                ps[:].rearrange("p (h e) -> p h e", e=64))

                    if sc == len(SCG) - 1:
                        # last chunk group: no further K/V projection exists
                        # to fill drain stalls, so retire the projection psum
                        # and run with double-buffered PV accumulators
                        aps.close()
                        psc_u = aps2.enter_context(
                            tc.tile_pool(name="psc2", bufs=2, space="PSUM"))
                        ppv_u = aps2.enter_context(
                            tc.tile_pool(name="ppv2", bufs=2, space="PSUM"))
                    else:
                        psc_u, ppv_u = psc, ppv

                    for p in range(8):
                        h0, h1 = 2 * p, 2 * p + 1
                        pva = ppv_u.tile([128, 2, SQ], F32, name="pva")
                        for i in range(ntc):
                            tci = sc * 4 + i
                            s01 = psc_u.tile([128, 2, SQ], F32, name="s01")
                            _mm(nc, s01[:, 0, :],
                                kT[0:64, p, i * 128:(i + 1) * 128],
                                qT[0:64, p, :], start=True, stop=True,
                                tile_position=(0, 0))
                            _mm(nc, s01[:, 1, :],
                                kT[64:128, p, i * 128:(i + 1) * 128],
                                qT[64:128, p, :], start=True, stop=True,
                                tile_position=(64, 0))
                            e01 = pexp.tile([128, 2, SQ], DT, name="e01")
                            nc.scalar.activation(e01[:], s01[:], AF.Exp,
                                                 bias=mask_sb[:, tci:tci + 1],
                                                 scale=0.125)
                            _mm(nc, pva[0:65, 0, :], vt[:, i, h0, :], e01[:, 0, :],
                                start=(i == 0), stop=(i == ntc - 1))
                            _mm(nc, pva[0:65, 1, :], vt[:, i, h1, :], e01[:, 1, :],
                                start=(i == 0), stop=(i == ntc - 1))
                        a0, c0 = 32 * (h0 % 4), h0 // 4
                        a1, c1 = 32 * (h1 % 4), h1 // 4
                        if sc == 0:
                            nc.vector.tensor_copy(acc[0:64, p, :], pva[0:64, 0, :])
                            nc.vector.tensor_copy(acc[64:128, p, :], pva[0:64, 1, :])
                            nc.vector.tensor_copy(nrm[a0:a0 + 1, c0, :], pva[64:65, 0, :])
                            nc.vector.tensor_copy(nrm[a1:a1 + 1, c1, :], pva[64:65, 1, :])
                        else:
                            nc.vector.tensor_add(acc[0:64, p, :],
                                                 _f(acc[0:64, p, :]), pva[0:64, 0, :])
                            nc.vector.tensor_add(acc[64:128, p, :],
                                                 _f(acc[64:128, p, :]), pva[0:64, 1, :])
                            nc.vector.tensor_add(nrm[a0:a0 + 1, c0, :],
                                                 nrm[a0:a0 + 1, c0, :], pva[64:65, 0, :])
                            nc.vector.tensor_add(nrm[a1:a1 + 1, c1, :],
                                                 nrm[a1:a1 + 1, c1, :], pva[64:65, 1, :])

                # normalize: acc[:, p, :] *= 1/nrm via selector-matmul bcast;
                # reciprocal per c-group so it pipelines with the tail PVs
                aps2.close()
                ppb = A.enter_context(tc.tile_pool(name="ppb", bufs=2, space="PSUM"))
                pnr = A.enter_context(tc.tile_pool(name="pnr", bufs=2))
                for c in range(4):
                    nc.vector.reciprocal_approx_fast(
                        out=nrm[:, c, :], in_=nrm[:, c, :])
                # pre-load the Sqrt ACT table set now (all attention exps are
                # done); keyed off the last reciprocal so the scheduler
                # cannot hoist the table swap above the exps
                scr = pnr.tile([1, 1], F32, name="scr")
                nc.scalar.activation(scr[:], nrm[0:1, 3, 0:1], AF.Sqrt)
                for p in range(8):
                    sel = sel_e if p % 2 == 0 else sel_o
                    ps_rb = ppb.tile([128, SQ], F32, name="ps_rb")
                    nc.tensor.matmul(ps_rb[:], sel, nrm[:, p // 2, :],
                                     start=True, stop=True)
                    nc.vector.tensor_mul(acc[:, p, :], _f(acc[:, p, :]), ps_rb[:])

            # ---- Stage 3: Wo + add1 + LN1 + residual ----
            with ExitStack() as st:
                pw = st.enter_context(tc.tile_pool(name="pw3", bufs=1))
                b1_sb = pw.tile([128, 8], F32, name="b1")
                g1_sb = pw.tile([128, 8], F32, name="g1")
                bb1_sb = pw.tile([128, 8], F32, name="bb1")
                if not triv:
                    nc.sync.dma_start(b1_sb[:], d_b1.ap())
                    nc.sync.dma_start(g1_sb[:], d_g1.ap())
                    nc.sync.dma_start(bb1_sb[:], d_bb1.ap())

                # x1 split into two tiles so FFN1 can start on the first half
                # while the LN apply chain still writes the second
                x1a = p_x1.tile([128, 4, SQ], DT, name="x1a")
                x1b = p_x1.tile([128, 4, SQ], DT, name="x1b")
                x1ap = [x1a[:, d, :] for d in range(4)] + \
                       [x1b[:, d, :] for d in range(4)]
                pao = st.enter_context(tc.tile_pool(name="pao", bufs=1))
                ao = pao.tile([128, 8, SQ], DT, name="ao")
                padd = st.enter_context(tc.tile_pool(name="padd1", bufs=1, space="PSUM"))
                with ExitStack() as stW:
                    pp = stW.enter_context(tc.tile_pool(name="pp3", bufs=2, space="PSUM"))
                    for do in range(8):
                        ps = pp.tile([128, SQ], F32, name="ps3a")
                        for d in range(8):
                            _mm(nc, ps[:], wo_sb[:, d, do * 128:(do + 1) * 128],
                                acc[:, d, :], start=(d == 0), stop=(d == 7))
                        nc.scalar.copy(ao[:, do, :], ps[:])
                # add1 in two 4-bank d-outer passes: pass MMs interleave with
                # the Wo do-loop above as its ao chunks land
                l1 = pao.tile([128, 8, SQ], DT, name="l1")
                for half in range(2):
                    aps4 = padd.tile([128, 4, SQ], F32, name="a1ps", tag="a1ps")
                    for d in range(8):
                        for j in range(4):
                            do = half * 4 + j
                            _mm(nc, aps4[:, j, :],
                                aw1_sb[:, d, do * 128:(do + 1) * 128],
                                ao[:, d, :], start=(d == 0), stop=(d == 7))
                    for j in range(4):
                        do = half * 4 + j
                        if triv:
                            eng = nc.scalar if j < 2 else nc.vector
                            if j < 2:
                                eng.copy(l1[:, do, :], aps4[:, j, :])
                            else:
                                eng.tensor_copy(l1[:, do, :], aps4[:, j, :])
                        else:
                            nc.vector.tensor_scalar(l1[:, do, :], aps4[:, j, :],
                                                    b1_sb[:, do:do + 1], None, OP.add)
                layernorm_block(st, l1, g1_sb, bb1_sb,
                                [xq2_sb[:, d, :] for d in range(8)], x1ap, "a")

        # ================= Stage 4: FFN + add2 + LN2 + residual =================
        with ExitStack() as st:
            pff = st.enter_context(tc.tile_pool(name="pff", bufs=1))
            ff = pff.tile([128, 8, SQ], DT, name="ff")
            aw2_sb = pff.tile([128, 8, D], DT, name="aw2")
            for do in range(8):
                nc.sync.dma_start(aw2_sb[:, :, do * 128:(do + 1) * 128],
                                  r_aw2[:, :, do * 128:(do + 1) * 128])
            padd2 = st.enter_context(tc.tile_pool(name="padd2", bufs=1, space="PSUM"))
            with ExitStack() as st4a:
                ph = st4a.enter_context(tc.tile_pool(name="ph", bufs=1))
                h_sb = ph.tile([128, 32, SQ], DT, name="h")
                pwc = st4a.enter_context(tc.tile_pool(name="pwc", bufs=6))
                pwc2 = st4a.enter_context(tc.tile_pool(name="pwc2", bufs=4))
                fb1_sb = ph.tile([128, 32], F32, name="fb1")
                fb2_sb = ph.tile([128, 8], F32, name="fb2")
                if not triv:
                    nc.sync.dma_start(fb1_sb[:], d_fb1.ap())
                    nc.sync.dma_start(fb2_sb[:], d_fb2.ap())
                pp = st4a.enter_context(tc.tile_pool(name="pp4", bufs=2, space="PSUM"))

                for f in range(32):
                    if f < 16:
                        w1t = fw1a[:, :, f * 128:(f + 1) * 128]
                    else:
                        w1c = pwc.tile([128, 8, 128], DT, name="w1c")
                        nc.sync.dma_start(w1c[:], r_fw1[:, :, f * 128:(f + 1) * 128])
                        w1t = w1c[:]
                    ps = pp.tile([128, SQ], F32, name="ps4a")
                    for d in range(8):
                        _mm(nc, ps[:], w1t[:, d, :], x1ap[d],
                            start=(d == 0), stop=(d == 7))
                    if triv:
                        nc.vector.tensor_scalar(h_sb[:, f, :], ps[:],
                                                0.0, None, OP.max)
                    else:
                        nc.vector.tensor_scalar(h_sb[:, f, :], ps[:],
                                                fb1_sb[:, f:f + 1], 0.0,
                                                OP.add, OP.max)

                for do in range(8):
                    w2t = pwc2.tile([128, 32, 128], DT, name="w2c")
                    nc.sync.dma_start(w2t[:], r_fw2[:, :, do * 128:(do + 1) * 128])
                    ps = pp.tile([128, SQ], F32, name="ps4b")
                    for f in range(32):
                        _mm(nc, ps[:], w2t[:, f, :], h_sb[:, f, :],
                            start=(f == 0), stop=(f == 31))
                    if triv:
                        nc.scalar.copy(ff[:, do, :], ps[:])
                    else:
                        nc.vector.tensor_scalar(ff[:, do, :], ps[:],
                                                fb2_sb[:, do:do + 1], None, OP.add)

            with ExitStack() as st4b:
                pw = st4b.enter_context(tc.tile_pool(name="pw4", bufs=1))
                b2_sb = pw.tile([128, 8], F32, name="b2")
                g2_sb = pw.tile([128, 8], F32, name="g2")
                bb2_sb = pw.tile([128, 8], F32, name="bb2")
                if not triv:
                    nc.sync.dma_start(b2_sb[:], d_b2.ap())
                    nc.sync.dma_start(g2_sb[:], d_g2.ap())
                    nc.sync.dma_start(bb2_sb[:], d_bb2.ap())

                # add2 in two 4-bank d-outer passes: pass-A MMs interleave
                # with the FFN2 do-loop as its ff chunks land
                l2 = pw.tile([128, 8, SQ], DT, name="l2")
                for half in range(2):
                    aps4 = padd2.tile([128, 4, SQ], F32, name="a2ps", tag="a2ps")
                    for d in range(8):
                        for j in range(4):
                            do = half * 4 + j
                            _mm(nc, aps4[:, j, :],
                                aw2_sb[:, d, do * 128:(do + 1) * 128],
                                ff[:, d, :], start=(d == 0), stop=(d == 7))
                    for j in range(4):
                        do = half * 4 + j
                        if triv:
                            if j < 2:
                                nc.scalar.copy(l2[:, do, :], aps4[:, j, :])
                            else:
                                nc.vector.tensor_copy(l2[:, do, :], aps4[:, j, :])
                        else:
                            nc.vector.tensor_scalar(l2[:, do, :], aps4[:, j, :],
                                                    b2_sb[:, do:do + 1], None, OP.add)

                outa = pw.tile([128, 4, SQ], DT, name="outa")
                outb = pw.tile([128, 4, SQ], DT, name="outb")
                outap = [outa[:, d, :] for d in range(4)] + \
                        [outb[:, d, :] for d in range(4)]
                layernorm_block(st4b, l2, g2_sb, bb2_sb, x1ap, outap, "b")
                nc.sync.dma_start(r_out[:, 0:4, :], outa[:])
                nc.sync.dma_start(r_out[:, 4:8, :], outb[:])

    nc.compile()
    return nc


_NC = {}


def _get_nc(nact, triv):
    key = (nact, triv, _MODE)
    if key not in _NC:
        _NC[key] = build_nc(nact, triv)
    return _NC[key]


def _prep_inputs(inputs):
    """Host-side shard prep: per-core input dicts."""
    x = np.asarray(inputs["batch_x"], np.float32)       # [B, S, D]
    lens = np.asarray(inputs["len_chair"], np.int64)
    wq = np.ascontiguousarray(
        np.asarray(inputs["Wq"], np.float32).transpose(1, 0, 2).reshape(D, D))
    wk = np.ascontiguousarray(
        np.asarray(inputs["Wk"], np.float32).transpose(1, 0, 2).reshape(D, D))
    wv = np.ascontiguousarray(
        np.asarray(inputs["Wv"], np.float32).transpose(1, 0, 2).reshape(D, D))
    com = {
        "wq": wq, "wk": wk, "wv": wv,
        "wo": np.ascontiguousarray(np.asarray(inputs["Wo"], np.float32)),
        "aw1": np.ascontiguousarray(np.asarray(inputs["add1_w"], np.float32)),
        "aw2": np.ascontiguousarray(np.asarray(inputs["add2_w"], np.float32)),
        "fw1": np.ascontiguousarray(np.asarray(inputs["ff_w1"], np.float32)),
        "fw2": np.ascontiguousarray(np.asarray(inputs["ff_w2"], np.float32)),
        "b1c": _chunk(inputs["add1_b"]), "g1c": _chunk(inputs["ln1_g"]),
        "bb1c": _chunk(inputs["ln1_b"]), "fb1c": _chunk(inputs["ff_b1"]),
        "fb2c": _chunk(inputs["ff_b2"]), "b2c": _chunk(inputs["add2_b"]),
        "g2c": _chunk(inputs["ln2_g"]), "bb2c": _chunk(inputs["ln2_b"]),
    }
    xT = [np.ascontiguousarray(x[b].T) for b in range(B)]   # [D, S]
    masks = []
    for b in range(B):
        m = np.where(np.arange(S) >= lens[b], np.float32(MASK_NEG),
                     np.float32(0.0)).astype(np.float32)
        masks.append(np.ascontiguousarray(m.reshape(S // 128, 128).T))
    in_maps = []
    for c in range(NCORES):
        b, q = c // 4, c % 4
        m = dict(com)
        m["xT"] = xT[b]
        m["xq"] = np.ascontiguousarray(xT[b][:, q * SQ:(q + 1) * SQ])
        m["maskb"] = masks[b]
        in_maps.append(m)
    return in_maps


def _chunk(v):
    v = np.asarray(v, np.float32)
    return np.ascontiguousarray(v.reshape(-1, 128).T)


DT_KEYS = ("xT", "xq", "wq", "wk", "wv", "wo", "aw1", "aw2", "fw1", "fw2")


def kernel(trace=False, **inputs):
    _install_ntff_hook()
    lens = np.asarray(inputs["len_chair"], np.int64)
    nact = int(max(1, min(S // 128, -(-int(lens.max()) // 128))))
    triv = (np.all(np.asarray(inputs["ln1_g"]) == 1.0)
            and np.all(np.asarray(inputs["ln2_g"]) == 1.0)
            and not np.any(np.asarray(inputs["ln1_b"]))
            and not np.any(np.asarray(inputs["ln2_b"]))
            and not np.any(np.asarray(inputs["add1_b"]))
            and not np.any(np.asarray(inputs["add2_b"]))
            and not np.any(np.asarray(inputs["ff_b1"]))
            and not np.any(np.asarray(inputs["ff_b2"])))
    nc = _get_nc(nact, triv)
    in_maps = _prep_inputs(inputs)
    import ml_dtypes
    np_dt = mybir.dt.np(DT)
    cache = {}

    def _cast(a, dtype):
        key = (id(a), np.dtype(dtype).str)
        if key not in cache:
            cache[key] = np.ascontiguousarray(a.astype(dtype))
        return cache[key]

    for m in in_maps:
        if np_dt != np.float32:
            for k in DT_KEYS:
                m[k] = _cast(m[k], np_dt)
    res = run_bass_kernel_spmd(nc, in_maps, core_ids=list(range(NCORES)),
                               trace=trace)
    out = np.empty((B, S, D), np.float32)
    for c in range(NCORES):
        b, q = c // 4, c % 4
        out[b, q * SQ:(q + 1) * SQ, :] = \
            np.asarray(res.results[c]["out"]).astype(np.float32).T
    kernel.last_exec_time_ns = res.exec_time_ns
    return out
